# revision 10
# baseline (speedup 1.0000x reference)
"""Trainium2 Bass kernel for nn_EnhancedTFNLayer (RBF field projection +
diffusion + sampling + LN/linear epilogue), data-parallel over batch on 8 cores.

Low-rank field pipeline (R=128 orthonormal basis Q fitted on host from the
parameter inputs only):

  phi[n, j] = exp(-(p_n - c_j)^2 / (2 s^2))   anchor features (fp16
              split-precision K=8 matmul + Exp)
  C = Wq^T (phi^T emb)                        field coords
  4x diffusion: C' = SLQ C + QTW @ tanh(Qsub^T (C W_int) + b_int)
              (tanh evaluated on a 256-point subsampled grid; QTW is a
               host-fitted quadrature back-projection, factor DT included)
  sampled = phi (MQ C)
  x = sampled + emb ; out = LN2(LN1(x) @ (W_out + I))   [residuals folded]

All matmul operands bf16 (emb converted on host); LN stats via stt/ts
accum_out (sum) + tensor_tensor_reduce (sum of squares); PSUM evacuations
spread across DVE / Pool / Act engines.
"""
import sys
import hashlib
import numpy as np

for _p in ("/opt/trn_rl_repo", "/root/.axon_site/_ro/trn_rl_repo"):
    if _p not in sys.path:
        sys.path.insert(0, _p)

import concourse.bass as bass
import concourse.bacc as bacc
import concourse.tile as tile
from concourse import mybir

F32 = mybir.dt.float32
BF16 = mybir.dt.bfloat16
FP16 = mybir.dt.float16
ACTF = mybir.ActivationFunctionType
ALU = mybir.AluOpType
AXL = mybir.AxisListType

B, N, G, D = 16, 4096, 1024, 256
NUM_STEPS, DT, EPS = 4, 0.01, 1e-5
R = 128
SSUB = 256               # tanh-subsampled grid points
NT = N // 128            # 32 token tiles per batch
BL = 2                   # batches per core
NCORES = 8

_CACHE = {}


def _bf16(x):
    x = np.ascontiguousarray(x, np.float32)
    u = x.view(np.uint32)
    r = ((u >> 16) + ((u >> 15) & 1)).astype(np.uint32) << 16
    return r.view(np.float32)


def _fp16(x):
    return np.float16(np.asarray(x, np.float64).astype(np.float32)).astype(np.float32)


# --------------------------------------------------------------------------
# host-side operator fitting (float64; parameter inputs only)
# --------------------------------------------------------------------------
def _host_plan(sigma, alpha, grid, W_int, b_int, W_out, b_out,
               ln1_g, ln1_b, ln2_g, ln2_b):
    rng = np.random.default_rng(0)
    c0 = 1.0 - 2.0 * alpha * DT
    c1 = alpha * DT
    pg = np.linspace(0.0, 1.0, 8193)
    K = np.exp(-((pg[:, None] - grid[None, :]) ** 2) / (2 * sigma * sigma))
    nsyn = 384
    sub = rng.choice(len(pg), size=256, replace=False)
    Fsyn = K[sub].T @ rng.standard_normal((256, nsyn))
    Fsyn /= np.abs(Fsyn).max(0, keepdims=True) + 1e-30
    fscale = np.sqrt(N * sigma * np.sqrt(np.pi))
    wnorm = np.linalg.norm(W_int, axis=0)
    wcols = rng.choice(len(wnorm), size=nsyn)
    gains = fscale * wnorm[wcols] * rng.uniform(0.5, 2.0, nsyn)
    Tsyn = np.tanh(Fsyn * gains[None, :])
    Msvd = np.concatenate([K, (Tsyn * 0.1).T], axis=0)
    _, _, Vt = np.linalg.svd(Msvd, full_matrices=False)
    Q = Vt[:R]                                            # [R, G] orthonormal
    # anchors
    c = np.linspace(-0.08, 1.08, R)
    s = 2.2 * (c[1] - c[0])
    F = np.exp(-((pg[:, None] - c[None, :]) ** 2) / (2 * s * s))
    Qk = K @ Q.T
    Wq, *_ = np.linalg.lstsq(F, Qk, rcond=1e-8)           # [R, R]
    Qt = Q.T
    LQt = c0 * Qt.copy()
    LQt[1:-1] += c1 * (Qt[:-2] + Qt[2:])
    LQt[0] += c1 * (Qt[0] + Qt[1])
    LQt[-1] += c1 * (Qt[-2] + Qt[-1])
    SLQ = Q @ LQt                                         # [R, R]
    u = pg * (G - 1)
    i0 = np.clip(np.floor(u), 0, G - 2).astype(int)
    w = u - i0
    lerpQ = Qt[i0] * (1 - w)[:, None] + Qt[i0 + 1] * w[:, None]
    MQ, *_ = np.linalg.lstsq(F, lerpQ, rcond=1e-5)        # [R, R]

    # subsampled-tanh quadrature back-projection QTW [R, SSUB]
    subidx = np.unique(np.linspace(0, G - 1, SSUB).round().astype(int))
    assert len(subidx) == SSUB
    nsyn2 = 1024
    Fg = np.exp(-((grid[:, None] - grid[None, ::8]) ** 2) / (2 * sigma * sigma))
    fields = Fg @ rng.standard_normal((Fg.shape[1], nsyn2))
    fields /= np.abs(fields).max(0, keepdims=True) + 1e-30
    gains2 = fscale * wnorm[rng.choice(len(wnorm), size=nsyn2)] * \
        np.exp(rng.uniform(np.log(0.25), np.log(4.0), nsyn2))
    TG = np.tanh(fields * gains2[None, :])                # [G, nsyn2]
    target = Q @ TG
    A = TG[subidx, :]
    lam = 1e-6 * np.linalg.norm(A) ** 2 / A.shape[0]
    QTW = np.linalg.solve(A @ A.T + lam * np.eye(SSUB), A @ target.T).T

    # fp16 split-precision anchor coefficient matrix [8, R]
    # pp8 rows on device: [qh, qh, qlr, ph, ph, pl, 1, 1]
    a3 = -1.0 / (2 * s * s)
    a1 = c / (s * s)
    a2 = -c * c / (2 * s * s)
    a3h = _fp16(a3); a3l = a3 - a3h
    a1h = _fp16(a1); a1l = a1 - a1h
    a2h = _fp16(a2); a2l = a2 - a2h
    anch8 = np.stack([
        np.full(R, a3h), np.full(R, a3l), np.full(R, a3 / 2048.0),
        a1h, a1l, a1 / 4096.0,
        a2h, a2l,
    ], axis=0)

    # affine folds: enh_aff = enh*g1 + b1 ; v = enh_aff @ (W_out + I) + b_out
    Wp = ln1_g[:, None] * (W_out + np.eye(D))             # rows scaled by g1
    brow = b_out + ln1_b @ (W_out + np.eye(D))            # const row
    f32 = lambda x: np.ascontiguousarray(x, dtype=np.float32)
    f16 = lambda x: np.ascontiguousarray(x, dtype=np.float16)

    # bf16 const blob [128, W] (values pre-rounded to bf16, stored as f32 on
    # host; device tile dtype BF16 so DMA converts? no -- DMA does not convert.
    # Host passes ml_dtypes.bfloat16 array instead; see _pack_bf16.)
    qsub = Q[:, subidx]                                   # [R, SSUB]
    qtw_t = (QTW * DT).T.reshape(2, 128, R).transpose(1, 0, 2)  # [128,2,R]
    wi = W_int.reshape(2, 128, D).transpose(1, 0, 2)      # [128,2,D]
    wo = Wp.reshape(2, 128, D).transpose(1, 0, 2)         # [128,2,D]
    cb = np.concatenate([
        qsub,                                             # [:,0:256]
        qtw_t.reshape(128, 2 * R),                        # [:,256:512]
        SLQ.T, Wq, MQ.T,                                  # 512:640,640:768,768:896
        wi.reshape(128, 2 * D),                           # 896:1408
        wo.reshape(128, 2 * D),                           # 1408:1920
        np.eye(128),                                      # 1920:2048
    ], axis=1)
    # row blob (bf16) [1, 512+]: bint row | brow | ones128
    crow = np.concatenate([
        b_int.reshape(1, D), brow.reshape(1, D), np.ones((1, 128)),
    ], axis=1)
    # f32 misc blob [128, 5]: epsb | g2? b2? (ln2 affine rows go separately)
    cg = np.full((128, 1), EPS)
    # ln2 affine rows [128, 2*D] f32 (only DMA'd/used when ln2_aff)
    caff = np.concatenate([np.broadcast_to(ln2_g, (128, D)),
                           np.broadcast_to(ln2_b, (128, D))], axis=1)

    import ml_dtypes
    bfl = lambda x: np.ascontiguousarray(x, dtype=ml_dtypes.bfloat16)
    consts = {
        "anch8": f16(anch8),
        "ones16": f16(np.ones((2, N))),
        "cb": bfl(cb),
        "crow": bfl(crow),
        "cg": f32(cg),
        "caff": f32(caff),
    }
    flags = {
        "use_bint": bool(np.any(b_int != 0)),
        "use_brow": bool(np.any(np.abs(brow) > 1e-12)),
        "ln2_aff": bool(np.any(ln2_g != 1) or np.any(ln2_b != 0)),
    }
    return consts, flags


# --------------------------------------------------------------------------
# device module
# --------------------------------------------------------------------------
def _build_module(flags, repeats=1, parts=("s1", "diff", "epi")):
    import os
    SAFE = os.environ.get("SAFE", "1") == "1"
    FEATS = set(os.environ.get("FEATS", "").split(","))
    nc = bacc.Bacc(trn_type="TRN2")
    emb_d = nc.dram_tensor("emb", [BL, N, D], BF16, kind="ExternalInput")
    pos_d = nc.dram_tensor("pos", [BL, N, 1], F32, kind="ExternalInput")
    const_specs = {
        "anch8": ([8, R], FP16),
        "ones16": ([2, N], FP16),
        "cb": ([128, 2048], BF16),
        "crow": ([1, 2 * D + 128], BF16),
        "cg": ([128, 1], F32),
        "caff": ([128, 2 * D], F32),
    }
    cd = {k: nc.dram_tensor(k, sh, dt, kind="ExternalInput")
          for k, (sh, dt) in const_specs.items()}
    out_d = nc.dram_tensor("out", [BL, N, D], BF16, kind="ExternalOutput")
    scratch_d = nc.dram_tensor("scratch", [BL, 4, N], FP16, kind="Internal")

    with tile.TileContext(nc) as tc:
        with tc.tile_pool(name="consts", bufs=1) as cp, \
             tc.tile_pool(name="emb", bufs=2) as embp, \
             tc.tile_pool(name="phi", bufs=2) as phip, \
             tc.tile_pool(name="coef", bufs=2) as coefp, \
             tc.tile_pool(name="pre", bufs=2) as prep, \
             tc.tile_pool(name="work", bufs=3) as wp, \
             tc.tile_pool(name="tiny", bufs=8) as tp, \
             tc.tile_pool(name="psB", bufs=1, space="PSUM") as psB:

            # ---- constants ----
            blob = {}
            for k, (sh, dt) in const_specs.items():
                if k == "caff" and not flags["ln2_aff"]:
                    continue
                blob[k] = cp.tile(sh, dt, tag=k, name=f"c_{k}")
                nc.sync.dma_start(blob[k][:], cd[k][tuple(slice(None) for _ in sh)])
            _cb = blob["cb"]
            ct = {
                "anch8": blob["anch8"],
                "qsub": _cb[:, 0:256],
                "qtw": _cb[:, 256:512].rearrange("p (a b) -> p a b", a=2),
                "slt": _cb[:, 512:640], "wq": _cb[:, 640:768],
                "mqt": _cb[:, 768:896],
                "wi": _cb[:, 896:1408].rearrange("p (a b) -> p a b", a=2),
                "wo": _cb[:, 1408:1920].rearrange("p (a b) -> p a b", a=2),
                "ident": _cb[:, 1920:2048],
                "bint_row": blob["crow"][:, 0:D],
                "brow": blob["crow"][:, D:2 * D],
                "ones1": blob["crow"][:, 2 * D:2 * D + 128],
                "epsb": blob["cg"][:, 0:1],
            }
            if flags["ln2_aff"]:
                ct["g2"] = blob["caff"][:, 0:D]
                ct["b2"] = blob["caff"][:, D:2 * D]

            import contextlib
            loopctx = tc.For_i(0, repeats, 1) if repeats > 1 else contextlib.nullcontext()
            with loopctx:
              st = [dict() for _ in range(BL)]

              def load_emb(b):
                  s = st[b]
                  s["emb"] = embp.tile([128, NT, D], BF16, tag="emb",
                                       name=f"emb_{b}")
                  eap = emb_d[b].rearrange("(t q) d -> q t d", q=128)
                  for k4 in range(4):
                      nc.sync.dma_start(s["emb"][:, 8 * k4:8 * (k4 + 1), :],
                                        eap[:, 8 * k4:8 * (k4 + 1), :])

              def prologue(b):
                  """fp16 split rows: scratch rows [ph, pl, qh, qlr]."""
                  s = st[b]
                  p16 = prep.tile([16, 256], F32, tag="p16", name=f"p16_{b}")
                  nc.sync.dma_start(p16[:],
                                    pos_d[b, :, 0].rearrange("(k j) -> k j", k=16))
                  ph = prep.tile([16, 256], FP16, tag="ph", name=f"ph_{b}")
                  nc.vector.tensor_copy(ph[:], p16[:])
                  dd = prep.tile([16, 256], F32, tag="dd", name=f"dd_{b}")
                  nc.vector.tensor_sub(dd[:], p16[:], ph[:])
                  pl = prep.tile([16, 256], FP16, tag="pl", name=f"pl_{b}")
                  nc.vector.tensor_scalar(pl[:], dd[:], 4096.0, None, op0=ALU.mult)
                  qq = prep.tile([16, 256], F32, tag="qq", name=f"qq_{b}")
                  nc.vector.tensor_mul(qq[:], ph[:], ph[:])
                  qh = prep.tile([16, 256], FP16, tag="qh", name=f"qh_{b}")
                  nc.vector.tensor_copy(qh[:], qq[:])
                  ee = prep.tile([16, 256], F32, tag="ee", name=f"ee_{b}")
                  nc.vector.tensor_sub(ee[:], qq[:], qh[:])
                  rr = prep.tile([16, 256], F32, tag="rr", name=f"rr_{b}")
                  nc.vector.tensor_mul(rr[:], ph[:], pl[:])
                  qlr = prep.tile([16, 256], FP16, tag="qlr", name=f"qlr_{b}")
                  nc.vector.scalar_tensor_tensor(qlr[:], ee[:], 2048.0, rr[:],
                                                 op0=ALU.mult, op1=ALU.add)
                  from concourse.tile_rust import add_dep_helper
                  iw = []
                  for i, t in enumerate((ph, pl, qh, qlr)):
                      iw.append(nc.sync.dma_start(
                          scratch_d[b, i].rearrange("(k j) -> k j", k=16), t[:]))
                  pp8 = prep.tile([8, N], FP16, tag="pp8", name=f"pp8_{b}")
                  rd = [(0, 2), (1, 2), (2, 3), (3, 0), (4, 0), (5, 1)]
                  for row, src in rd:
                      ir = nc.sync.dma_start(
                          pp8[row:row + 1, :],
                          scratch_d[b, src].rearrange("(one n) -> one n", one=1))
                      add_dep_helper(ir.ins, iw[src].ins, sync=True,
                                     reason="scratch RAW")
                  nc.sync.dma_start(pp8[6:8, :], cd["ones16"][:, :])
                  s["pp8"] = pp8

              def stage1_init(b):
                  s = st[b]
                  phiT = phip.tile([R, 8, 512], BF16, tag="phiT", name=f"phiT_{b}")
                  phiN = phip.tile([128, NT, 128], BF16, tag="phiN",
                                   name=f"phiN_{b}")
                  s["phiT"], s["phiN"] = phiT, phiN
                  s["pCt"] = psB.tile([128, 2, 256], F32, tag="ps2", bufs=6,
                                      name=f"pC_{b}")

              def stage1_chunk(b, j):
                  s = st[b]
                  pp8, emb_sb = s["pp8"], s["emb"]
                  phiT, phiN = s["phiT"], s["phiN"]
                  pC = s["pCt"][:, 0, :]
                  if True:
                      psPhi = psB.tile([128, 2, 256], F32, tag="ps2", bufs=6,
                                       name=f"psPhi_{b}_{j}")
                      psPhiv = psPhi[:].rearrange("p a b -> p (a b)")
                      nc.tensor.matmul(psPhiv, ct["anch8"][:, :],
                                       pp8[:, 512 * j:512 * (j + 1)],
                                       start=True, stop=True)
                      nc.scalar.activation(phiT[:, j, :], psPhiv, ACTF.Exp)
                      ptT = psB.tile([128, 512], BF16, tag="psbf", bufs=2,
                                     name=f"ptT_{b}_{j}")
                      for h in range(4):
                          nc.tensor.transpose(ptT[:, 128 * h:128 * (h + 1)],
                                              phiT[:, j, 128 * h:128 * (h + 1)],
                                              ct["ident"][:, :])
                      # evac: alternate DVE / Act (Pool cannot read PSUM)
                      dst = phiN[:, 4 * j:4 * (j + 1), :].rearrange("p a b -> p (a b)")
                      if j % 2 == 0:
                          nc.vector.tensor_copy(dst, ptT[:])
                      else:
                          nc.scalar.copy(dst, ptT[:])
                      for h in range(4):
                          t = 4 * j + h
                          nc.tensor.matmul(pC, phiN[:, t, :], emb_sb[:, t, :],
                                           start=(t == 0), stop=(t == NT - 1))

              def stage1_fin(b):
                  s = st[b]
                  pC = s["pCt"][:, 0, :]
                  craw = coefp.tile([R, D], BF16, tag="craw", name=f"craw_{b}")
                  nc.scalar.copy(craw[:], pC)
                  pC2t = psB.tile([128, 2, 256], F32, tag="ps2", bufs=6,
                                  name=f"pC2_{b}")
                  pC2 = pC2t[:, 0, :]
                  nc.tensor.matmul(pC2, ct["wq"][:, :], craw[:],
                                   start=True, stop=True)
                  C = coefp.tile([R, D], BF16, tag="C", bufs=4, name=f"C_{b}")
                  nc.vector.tensor_copy(C[:], pC2)
                  s["C"] = C

              def diffuse_step(b, step):
                  s = st[b]
                  C = s["C"]
                  ptC = psB.tile([128, 512], BF16, tag="psbf", bufs=2,
                                 name=f"ptC_{b}_{step}")
                  for h in range(2):
                      nc.tensor.transpose(ptC[:, 128 * h:128 * (h + 1)],
                                          C[:, 128 * h:128 * (h + 1)],
                                          ct["ident"][:, :])
                  Ct = wp.tile([128, 2, 128], BF16, tag="Ct", name=f"Ct_{b}_{step}")
                  nc.vector.tensor_copy(
                      Ct[:].rearrange("p a b -> p (a b)"), ptC[:, 0:256])
                  pCWt = psB.tile([128, 2, 256], F32, tag="ps2", bufs=6,
                                  name=f"pCW_{b}_{step}")
                  pCW = pCWt[:, 0, :]
                  for h in range(2):
                      nc.tensor.matmul(pCW, Ct[:, h, :], ct["wi"][:, h, :],
                                       start=(h == 0), stop=(h == 1))
                  CWb = wp.tile([R, D], BF16, tag="CWb", name=f"CWb_{b}_{step}")
                  nc.scalar.copy(CWb[:], pCW)
                  psF = psB.tile([128, 2, 256], F32, tag="ps2", bufs=6,
                                 name=f"psF_{b}_{step}")
                  for sc in range(2):
                      nc.tensor.matmul(psF[:, sc, :],
                                       ct["qsub"][:, 128 * sc:128 * (sc + 1)],
                                       CWb[:], start=True,
                                       stop=not flags["use_bint"])
                      if flags["use_bint"]:
                          nc.tensor.matmul(psF[:, sc, :], ct["ones1"][0:1, :],
                                           ct["bint_row"][0:1, :],
                                           start=False, stop=True)
                  T = wp.tile([128, 2, 256], BF16, tag="T", name=f"T_{b}_{step}")
                  nc.scalar.activation(T[:].rearrange("p a b -> p (a b)"),
                                       psF[:].rearrange("p a b -> p (a b)"),
                                       ACTF.Tanh)
                  pCnt = psB.tile([128, 2, 256], F32, tag="ps2", bufs=6,
                                  name=f"pCn_{b}_{step}")
                  pCn = pCnt[:, 0, :]
                  nc.tensor.matmul(pCn, ct["slt"][:, :], C[:],
                                   start=True, stop=False)
                  for sc in range(2):
                      nc.tensor.matmul(pCn, ct["qtw"][:, sc, :], T[:, sc, :],
                                       start=False, stop=(sc == 1))
                  C2 = coefp.tile([R, D], BF16, tag="C", bufs=4,
                                  name=f"C_{b}_{step}")
                  nc.vector.tensor_copy(C2[:], pCn)
                  s["C"] = C2

              def finish_coef(b):
                  s = st[b]
                  pMCt = psB.tile([128, 2, 256], F32, tag="ps2", bufs=6,
                                  name=f"pMC_{b}")
                  pMC = pMCt[:, 0, :]
                  nc.tensor.matmul(pMC, ct["mqt"][:, :], s["C"][:],
                                   start=True, stop=True)
                  MC = coefp.tile([R, D], BF16, tag="MC", name=f"MC_{b}")
                  nc.scalar.copy(MC[:], pMC)
                  s["MC"] = MC

              def epilogue_pair(b, p):
                  """tiles t0=2p, t1=2p+1"""
                  s = st[b]
                  phiT, phiN, MC, emb_sb = s["phiT"], s["phiN"], s["MC"], s["emb"]
                  invD = 1.0 / D
                  psX = psB.tile([128, 2, 256], F32, tag="ps2", bufs=6,
                                 name=f"psX_{b}_{p}")
                  for tp in range(2):
                      t = 2 * p + tp
                      j, h = divmod(t, 4)
                      nc.tensor.matmul(psX[:, tp, :],
                                       phiT[:, j, 128 * h:128 * (h + 1)],
                                       MC[:], start=True, stop=True)
                  x_bf = wp.tile([128, 2, 256], BF16, tag="x", bufs=4,
                                 name=f"x_{b}_{p}")
                  sx = tp_.tile([128, 4], F32, tag="sx", name=f"sx_{b}_{p}")
                  mv = tp_.tile([128, 4], F32, tag="mv", name=f"mv_{b}_{p}")
                  rst = tp_.tile([128, 2], F32, tag="rst", name=f"rst_{b}_{p}")
                  if SAFE:
                      for tp in range(2):
                          t = 2 * p + tp
                          if "sttacc" in FEATS:
                              nc.vector.scalar_tensor_tensor(
                                  x_bf[:, tp, :], psX[:, tp, :], 1.0,
                                  emb_sb[:, t, :], op0=ALU.mult, op1=ALU.add,
                                  accum_out=sx[:, tp:tp + 1])
                          else:
                              nc.vector.scalar_tensor_tensor(
                                  x_bf[:, tp, :], psX[:, tp, :], 1.0,
                                  emb_sb[:, t, :], op0=ALU.mult, op1=ALU.add)
                      if "ttr" in FEATS:
                          junkt = wp.tile([128, 2, 256], BF16, tag="junk", bufs=2,
                                          name=f"junkt_{b}_{p}")
                          for tp in range(2):
                              nc.vector.tensor_tensor_reduce(
                                  junkt[:, tp, :], x_bf[:, tp, :], x_bf[:, tp, :],
                                  1.0, 0.0, op0=ALU.mult, op1=ALU.add,
                                  accum_out=sx[:, 2 + tp:3 + tp])
                      for tp in range(2):
                          bn = tp_.tile([128, 6], F32, tag="bn",
                                        name=f"bn_{b}_{p}_{tp}")
                          nc.vector.bn_stats(bn[:], x_bf[:, tp, :])
                          nc.vector.bn_aggr(mv[:, 2 * tp:2 * tp + 2], bn[:])
                      for tp in range(2):
                          nc.scalar.activation(rst[:, tp:tp + 1],
                                               mv[:, 2 * tp + 1:2 * tp + 2],
                                               ACTF.Sqrt, bias=ct["epsb"][:, :])
                      nc.vector.reciprocal(rst[:], rst[:])
                  else:
                      for tp in range(2):
                          t = 2 * p + tp
                          nc.vector.scalar_tensor_tensor(x_bf[:, tp, :], psX[:, tp, :],
                                                         1.0, emb_sb[:, t, :],
                                                         op0=ALU.mult, op1=ALU.add,
                                                         accum_out=sx[:, tp:tp + 1])
                      junk = wp.tile([128, 2, 256], BF16, tag="junk", bufs=2,
                                     name=f"junk_{b}_{p}")
                      for tp in range(2):
                          nc.vector.tensor_tensor_reduce(
                              junk[:, tp, :], x_bf[:, tp, :], x_bf[:, tp, :],
                              1.0, 0.0, op0=ALU.mult, op1=ALU.add,
                              accum_out=sx[:, 2 + tp:3 + tp])
                      for tp in range(2):
                          nc.vector.tensor_scalar(mv[:, tp:tp + 1], sx[:, tp:tp + 1],
                                                  invD, None, op0=ALU.mult)
                          nc.vector.tensor_mul(mv[:, 2 + tp:3 + tp],
                                               mv[:, tp:tp + 1], mv[:, tp:tp + 1])
                          nc.vector.scalar_tensor_tensor(
                              mv[:, 2 + tp:3 + tp], sx[:, 2 + tp:3 + tp], invD,
                              mv[:, 2 + tp:3 + tp], op0=ALU.mult, op1=ALU.subtract)
                          nc.scalar.activation(rst[:, tp:tp + 1],
                                               mv[:, 2 + tp:3 + tp],
                                               ACTF.Sqrt, bias=ct["epsb"][:, :])
                      nc.vector.reciprocal(rst[:], rst[:])
                  enh = wp.tile([128, 2, 256], BF16, tag="enh", bufs=4,
                                name=f"enh_{b}_{p}")
                  ptE = psB.tile([128, 512], BF16, tag="psbf", bufs=2,
                                 name=f"ptE_{b}_{p}")
                  for tp in range(2):
                      mu_ap = mv[:, 2 * tp:2 * tp + 1] if SAFE else mv[:, tp:tp + 1]
                      eng = nc.gpsimd if (not SAFE or "poolts" in FEATS) else nc.vector
                      eng.tensor_scalar(enh[:, tp, :], x_bf[:, tp, :],
                                        mu_ap, rst[:, tp:tp + 1],
                                        op0=ALU.subtract, op1=ALU.mult)
                      for h2 in range(2):
                          nc.tensor.transpose(
                              ptE[:, 256 * tp + 128 * h2:256 * tp + 128 * (h2 + 1)],
                              enh[:, tp, 128 * h2:128 * (h2 + 1)],
                              ct["ident"][:, :])
                  enhT = wp.tile([128, 4, 128], BF16, tag="enhT", bufs=4,
                                 name=f"enhT_{b}_{p}")
                  nc.vector.tensor_copy(enhT[:].rearrange("p a b -> p (a b)"),
                                        ptE[:])
                  psV = psB.tile([128, 2, 256], F32, tag="ps2", bufs=6,
                                 name=f"psV_{b}_{p}")
                  for tp in range(2):
                      for h2 in range(2):
                          nc.tensor.matmul(psV[:, tp, :], enhT[:, 2 * tp + h2, :],
                                           ct["wo"][:, h2, :],
                                           start=(h2 == 0),
                                           stop=(h2 == 1 and not flags["use_brow"]))
                      if flags["use_brow"]:
                          nc.tensor.matmul(psV[:, tp, :], ct["ones1"][0:1, :],
                                           ct["brow"][0:1, :],
                                           start=False, stop=True)
                  v_bf = wp.tile([128, 2, 256], BF16, tag="v", bufs=4,
                                 name=f"v_{b}_{p}")
                  sv = tp_.tile([128, 4], F32, tag="sv", name=f"sv_{b}_{p}")
                  mv2 = tp_.tile([128, 4], F32, tag="mv2", name=f"mv2_{b}_{p}")
                  rst2 = tp_.tile([128, 2], F32, tag="rst2", name=f"rst2_{b}_{p}")
                  if SAFE:
                      for tp in range(2):
                          if "tsacc" in FEATS:
                              nc.vector.tensor_scalar(v_bf[:, tp, :], psV[:, tp, :],
                                                      1.0, 0.0, op0=ALU.mult,
                                                      op1=ALU.add,
                                                      accum_out=sv[:, tp:tp + 1])
                          else:
                              nc.vector.tensor_copy(v_bf[:, tp, :], psV[:, tp, :])
                      for tp in range(2):
                          bn2 = tp_.tile([128, 6], F32, tag="bn2",
                                         name=f"bn2_{b}_{p}_{tp}")
                          nc.vector.bn_stats(bn2[:], v_bf[:, tp, :])
                          nc.vector.bn_aggr(mv2[:, 2 * tp:2 * tp + 2], bn2[:])
                      for tp in range(2):
                          nc.scalar.activation(rst2[:, tp:tp + 1],
                                               mv2[:, 2 * tp + 1:2 * tp + 2],
                                               ACTF.Sqrt, bias=ct["epsb"][:, :])
                      nc.vector.reciprocal(rst2[:], rst2[:])
                  else:
                      for tp in range(2):
                          nc.vector.tensor_scalar(v_bf[:, tp, :], psV[:, tp, :],
                                                  1.0, 0.0, op0=ALU.mult,
                                                  op1=ALU.add,
                                                  accum_out=sv[:, tp:tp + 1])
                      junk2 = wp.tile([128, 2, 256], BF16, tag="junk2", bufs=2,
                                      name=f"junk2_{b}_{p}")
                      for tp in range(2):
                          nc.vector.tensor_tensor_reduce(
                              junk2[:, tp, :], v_bf[:, tp, :], v_bf[:, tp, :],
                              1.0, 0.0, op0=ALU.mult, op1=ALU.add,
                              accum_out=sv[:, 2 + tp:3 + tp])
                      for tp in range(2):
                          nc.vector.tensor_scalar(mv2[:, tp:tp + 1], sv[:, tp:tp + 1],
                                                  invD, None, op0=ALU.mult)
                          nc.vector.tensor_mul(mv2[:, 2 + tp:3 + tp],
                                               mv2[:, tp:tp + 1], mv2[:, tp:tp + 1])
                          nc.vector.scalar_tensor_tensor(
                              mv2[:, 2 + tp:3 + tp], sv[:, 2 + tp:3 + tp], invD,
                              mv2[:, 2 + tp:3 + tp], op0=ALU.mult, op1=ALU.subtract)
                          nc.scalar.activation(rst2[:, tp:tp + 1],
                                               mv2[:, 2 + tp:3 + tp],
                                               ACTF.Sqrt, bias=ct["epsb"][:, :])
                      nc.vector.reciprocal(rst2[:], rst2[:])
                  ot = wp.tile([128, 2, 256], BF16, tag="ot", bufs=3,
                               name=f"ot_{b}_{p}")
                  for tp in range(2):
                      mu_ap = mv2[:, 2 * tp:2 * tp + 1] if SAFE else mv2[:, tp:tp + 1]
                      eng = nc.gpsimd if (not SAFE or "poolts" in FEATS) else nc.vector
                      eng.tensor_scalar(ot[:, tp, :], v_bf[:, tp, :],
                                        mu_ap, rst2[:, tp:tp + 1],
                                        op0=ALU.subtract, op1=ALU.mult)
                      if flags["ln2_aff"]:
                          nc.vector.tensor_mul(ot[:, tp, :], ot[:, tp, :],
                                               ct["g2"][:, :])
                          nc.vector.tensor_add(ot[:, tp, :], ot[:, tp, :],
                                               ct["b2"][:, :])
                  nc.sync.dma_start(
                      out_d[b].rearrange("(t q) d -> q t d", q=128)
                           [:, 2 * p:2 * (p + 1), :], ot[:])

              tp_ = tp
              # ---- emission: interleave the two batches at fine grain ----
              for b in range(BL):
                  prologue(b)
              for b in range(BL):
                  load_emb(b)
              if "s1" in parts:
                  for b in range(BL):
                      stage1_init(b)
                  for j in range(8):
                      for b in range(BL):
                          stage1_chunk(b, j)
                  for b in range(BL):
                      stage1_fin(b)
                  if "diff" in parts:
                      for step in range(NUM_STEPS):
                          for b in range(BL):
                              diffuse_step(b, step)
                  for b in range(BL):
                      finish_coef(b)
                  if "epi" in parts:
                      for p in range(NT // 2):
                          for b in range(BL):
                              epilogue_pair(b, p)

    nc.compile()
    return nc


# --------------------------------------------------------------------------
# runner (same multi-core pjrt path as before)
# --------------------------------------------------------------------------
def _make_runner(nc):
    import jax
    import numpy as _np
    from jax.sharding import Mesh, PartitionSpec
    from jax.experimental.shard_map import shard_map
    from concourse import mybir as _mb
    from concourse.bass2jax import (install_neuronx_cc_hook, _bass_exec_p,
                                    partition_id_tensor)
    install_neuronx_cc_hook()
    partition_name = nc.partition_id_tensor.name if nc.partition_id_tensor else None
    in_names, out_names, out_avals, zero_outs = [], [], [], []
    for alloc in nc.m.functions[0].allocations:
        if not isinstance(alloc, _mb.MemoryLocationSet):
            continue
        name = alloc.memorylocations[0].name
        if alloc.kind == "ExternalInput":
            if name != partition_name:
                in_names.append(name)
        elif alloc.kind == "ExternalOutput":
            npdt = _mb.dt.np(alloc.dtype)
            out_names.append(name)
            out_avals.append(jax.core.ShapedArray(tuple(alloc.tensor_shape), npdt))
            zero_outs.append(_np.zeros(tuple(alloc.tensor_shape), npdt))
    n_params = len(in_names)
    n_outs = len(out_names)
    all_in = in_names + out_names + ([partition_name] if partition_name else [])

    def _body(*args):
        operands = list(args)
        if partition_name is not None:
            operands.append(partition_id_tensor())
        return tuple(_bass_exec_p.bind(
            *operands, out_avals=tuple(out_avals),
            in_names=tuple(all_in), out_names=tuple(out_names),
            lowering_input_output_aliases=(), sim_require_finite=True,
            sim_require_nnan=True, nc=nc))

    devices = jax.devices()[:NCORES]
    mesh = Mesh(_np.asarray(devices), ("core",))
    donate = tuple(range(n_params, n_params + n_outs))
    sharded = jax.jit(
        shard_map(_body, mesh=mesh,
                  in_specs=(PartitionSpec("core"),) * (n_params + n_outs),
                  out_specs=(PartitionSpec("core"),) * n_outs,
                  check_rep=False),
        donate_argnums=donate, keep_unused=True)

    def run(in_maps):
        per_core = [[_np.asarray(m[name]) for name in in_names] for m in in_maps]
        concat_in = [_np.concatenate([per_core[c][i] for c in range(NCORES)], axis=0)
                     for i in range(n_params)]
        concat_zero = [_np.zeros((NCORES * z.shape[0], *z.shape[1:]), z.dtype)
                       for z in zero_outs]
        outs = sharded(*concat_in, *concat_zero)
        outs = [_np.asarray(o) for o in outs]
        return {name: outs[i] for i, name in enumerate(out_names)}

    return run


def kernel(**inputs):
    import ml_dtypes
    emb = np.ascontiguousarray(inputs["embeddings"], dtype=np.float32)
    pos = np.ascontiguousarray(inputs["positions"], dtype=np.float32)
    grid = np.asarray(inputs["grid_points"], np.float64)[0, :, 0]
    params = dict(
        sigma=float(np.asarray(inputs["sigma"])),
        alpha=float(np.asarray(inputs["alpha"])),
        grid=grid,
        W_int=np.asarray(inputs["W_int"], np.float64),
        b_int=np.asarray(inputs["b_int"], np.float64),
        W_out=np.asarray(inputs["W_out"], np.float64),
        b_out=np.asarray(inputs["b_out"], np.float64),
        ln1_g=np.asarray(inputs["ln1_g"], np.float64),
        ln1_b=np.asarray(inputs["ln1_b"], np.float64),
        ln2_g=np.asarray(inputs["ln2_g"], np.float64),
        ln2_b=np.asarray(inputs["ln2_b"], np.float64),
    )
    key = hashlib.sha256(b"".join(np.asarray(v).tobytes() for v in params.values())).hexdigest()
    if key not in _CACHE:
        consts, flags = _host_plan(**params)
        nc = _build_module(flags)
        _CACHE[key] = (_make_runner(nc), consts)
    run, consts = _CACHE[key]

    embb = emb.astype(ml_dtypes.bfloat16)
    in_maps = []
    for c in range(NCORES):
        m = {"emb": embb[BL * c:BL * (c + 1)],
             "pos": pos[BL * c:BL * (c + 1)]}
        m.update(consts)
        in_maps.append(m)
    outs = run(in_maps)
    return np.ascontiguousarray(outs["out"].astype(np.float32))


# revision 12
# speedup vs baseline: 1.3999x; 1.3999x over previous
"""Trainium2 Bass kernel for nn_EnhancedTFNLayer (RBF field projection +
diffusion + sampling + LN/linear epilogue), data-parallel over batch on 8 cores.

Low-rank field pipeline (R=128 orthonormal basis Q fitted on host from the
parameter inputs only):

  phi[n, j] = exp(-(p_n - c_j)^2 / (2 s^2))   anchor features (fp16
              split-precision K=8 matmul + Exp)
  C = Wq^T (phi^T emb)                        field coords
  4x diffusion: C' = SLQ C + QTW @ tanh(Qsub^T (C W_int) + b_int)
              (tanh evaluated on a 256-point subsampled grid; QTW is a
               host-fitted quadrature back-projection, factor DT included)
  sampled = phi (MQ C)
  x = sampled + emb ; out = LN2(LN1(x) @ (W_out + I))   [residuals folded]

All matmul operands bf16 (emb converted on host); LN stats via stt/ts
accum_out (sum) + tensor_tensor_reduce (sum of squares); PSUM evacuations
spread across DVE / Pool / Act engines.
"""
import sys
import hashlib
import numpy as np

for _p in ("/opt/trn_rl_repo", "/root/.axon_site/_ro/trn_rl_repo"):
    if _p not in sys.path:
        sys.path.insert(0, _p)

import concourse.bass as bass
import concourse.bacc as bacc
import concourse.tile as tile
from concourse import mybir

F32 = mybir.dt.float32
BF16 = mybir.dt.bfloat16
FP16 = mybir.dt.float16
ACTF = mybir.ActivationFunctionType
ALU = mybir.AluOpType
AXL = mybir.AxisListType

B, N, G, D = 16, 4096, 1024, 256
NUM_STEPS, DT, EPS = 4, 0.01, 1e-5
R = 128
SSUB = 256               # tanh-subsampled grid points
NT = N // 128            # 32 token tiles per batch
BL = 2                   # batches per core
NCORES = 8

_CACHE = {}


def _bf16(x):
    x = np.ascontiguousarray(x, np.float32)
    u = x.view(np.uint32)
    r = ((u >> 16) + ((u >> 15) & 1)).astype(np.uint32) << 16
    return r.view(np.float32)


def _fp16(x):
    return np.float16(np.asarray(x, np.float64).astype(np.float32)).astype(np.float32)


# --------------------------------------------------------------------------
# host-side operator fitting (float64; parameter inputs only)
# --------------------------------------------------------------------------
def _host_plan(sigma, alpha, grid, W_int, b_int, W_out, b_out,
               ln1_g, ln1_b, ln2_g, ln2_b):
    rng = np.random.default_rng(0)
    c0 = 1.0 - 2.0 * alpha * DT
    c1 = alpha * DT
    pg = np.linspace(0.0, 1.0, 8193)
    K = np.exp(-((pg[:, None] - grid[None, :]) ** 2) / (2 * sigma * sigma))
    nsyn = 384
    sub = rng.choice(len(pg), size=256, replace=False)
    Fsyn = K[sub].T @ rng.standard_normal((256, nsyn))
    Fsyn /= np.abs(Fsyn).max(0, keepdims=True) + 1e-30
    fscale = np.sqrt(N * sigma * np.sqrt(np.pi))
    wnorm = np.linalg.norm(W_int, axis=0)
    wcols = rng.choice(len(wnorm), size=nsyn)
    gains = fscale * wnorm[wcols] * rng.uniform(0.5, 2.0, nsyn)
    Tsyn = np.tanh(Fsyn * gains[None, :])
    Msvd = np.concatenate([K, (Tsyn * 0.1).T], axis=0)
    _, _, Vt = np.linalg.svd(Msvd, full_matrices=False)
    Q = Vt[:R]                                            # [R, G] orthonormal
    # anchors
    c = np.linspace(-0.08, 1.08, R)
    s = 2.2 * (c[1] - c[0])
    F = np.exp(-((pg[:, None] - c[None, :]) ** 2) / (2 * s * s))
    Qk = K @ Q.T
    Wq, *_ = np.linalg.lstsq(F, Qk, rcond=1e-8)           # [R, R]
    Qt = Q.T
    LQt = c0 * Qt.copy()
    LQt[1:-1] += c1 * (Qt[:-2] + Qt[2:])
    LQt[0] += c1 * (Qt[0] + Qt[1])
    LQt[-1] += c1 * (Qt[-2] + Qt[-1])
    SLQ = Q @ LQt                                         # [R, R]
    u = pg * (G - 1)
    i0 = np.clip(np.floor(u), 0, G - 2).astype(int)
    w = u - i0
    lerpQ = Qt[i0] * (1 - w)[:, None] + Qt[i0 + 1] * w[:, None]
    MQ, *_ = np.linalg.lstsq(F, lerpQ, rcond=1e-5)        # [R, R]

    # subsampled-tanh quadrature back-projection QTW [R, SSUB]
    subidx = np.unique(np.linspace(0, G - 1, SSUB).round().astype(int))
    assert len(subidx) == SSUB
    nsyn2 = 1024
    Fg = np.exp(-((grid[:, None] - grid[None, ::8]) ** 2) / (2 * sigma * sigma))
    fields = Fg @ rng.standard_normal((Fg.shape[1], nsyn2))
    fields /= np.abs(fields).max(0, keepdims=True) + 1e-30
    gains2 = fscale * wnorm[rng.choice(len(wnorm), size=nsyn2)] * \
        np.exp(rng.uniform(np.log(0.25), np.log(4.0), nsyn2))
    TG = np.tanh(fields * gains2[None, :])                # [G, nsyn2]
    target = Q @ TG
    A = TG[subidx, :]
    lam = 1e-6 * np.linalg.norm(A) ** 2 / A.shape[0]
    QTW = np.linalg.solve(A @ A.T + lam * np.eye(SSUB), A @ target.T).T

    # fp16 split-precision anchor coefficient matrix [8, R]
    # pp8 rows on device: [qh, qh, qlr, ph, ph, pl, 1, 1]
    a3 = -1.0 / (2 * s * s)
    a1 = c / (s * s)
    a2 = -c * c / (2 * s * s)
    a3h = _fp16(a3); a3l = a3 - a3h
    a1h = _fp16(a1); a1l = a1 - a1h
    a2h = _fp16(a2); a2l = a2 - a2h
    anch8 = np.stack([
        np.full(R, a3h), np.full(R, a3l), np.full(R, a3 / 2048.0),
        a1h, a1l, a1 / 4096.0,
        a2h, a2l,
    ], axis=0)

    # affine folds: enh_aff = enh*g1 + b1 ; v = enh_aff @ (W_out + I) + b_out
    Wp = ln1_g[:, None] * (W_out + np.eye(D))             # rows scaled by g1
    brow = b_out + ln1_b @ (W_out + np.eye(D))            # const row
    f32 = lambda x: np.ascontiguousarray(x, dtype=np.float32)
    f16 = lambda x: np.ascontiguousarray(x, dtype=np.float16)

    # bf16 const blob [128, W] (values pre-rounded to bf16, stored as f32 on
    # host; device tile dtype BF16 so DMA converts? no -- DMA does not convert.
    # Host passes ml_dtypes.bfloat16 array instead; see _pack_bf16.)
    qsub = Q[:, subidx]                                   # [R, SSUB]
    qtw_t = (QTW * DT).T.reshape(2, 128, R).transpose(1, 0, 2)  # [128,2,R]
    wi = W_int.reshape(2, 128, D).transpose(1, 0, 2)      # [128,2,D]
    wo = Wp.reshape(2, 128, D).transpose(1, 0, 2)         # [128,2,D]
    cb = np.concatenate([
        qsub,                                             # [:,0:256]
        qtw_t.reshape(128, 2 * R),                        # [:,256:512]
        SLQ.T, Wq, MQ.T,                                  # 512:640,640:768,768:896
        wi.reshape(128, 2 * D),                           # 896:1408
        wo.reshape(128, 2 * D),                           # 1408:1920
        np.eye(128),                                      # 1920:2048
    ], axis=1)
    # row blob (bf16) [1, 512+]: bint row | brow | ones128
    crow = np.concatenate([
        b_int.reshape(1, D), brow.reshape(1, D), np.ones((1, 128)),
    ], axis=1)
    # f32 misc blob [128, 5]: epsb | g2? b2? (ln2 affine rows go separately)
    cg = np.full((128, 1), EPS)
    # ln2 affine rows [128, 2*D] f32 (only DMA'd/used when ln2_aff)
    caff = np.concatenate([np.broadcast_to(ln2_g, (128, D)),
                           np.broadcast_to(ln2_b, (128, D))], axis=1)

    import ml_dtypes
    bfl = lambda x: np.ascontiguousarray(x, dtype=ml_dtypes.bfloat16)
    consts = {
        "anch8": f16(anch8),
        "ones16": f16(np.ones((2, N))),
        "cb": bfl(cb),
        "crow": bfl(crow),
        "cg": f32(cg),
        "caff": f32(caff),
    }
    flags = {
        "use_bint": bool(np.any(b_int != 0)),
        "use_brow": bool(np.any(np.abs(brow) > 1e-12)),
        "ln2_aff": bool(np.any(ln2_g != 1) or np.any(ln2_b != 0)),
    }
    return consts, flags


# --------------------------------------------------------------------------
# device module
# --------------------------------------------------------------------------
def _build_module(flags, repeats=1, parts=("s1", "diff", "epi")):
    import os
    SAFE = os.environ.get("SAFE", "1") == "1"
    FEATS = set(os.environ.get("FEATS", "").split(","))
    nc = bacc.Bacc(trn_type="TRN2")
    emb_d = nc.dram_tensor("emb", [BL, N, D], BF16, kind="ExternalInput")
    pos_d = nc.dram_tensor("pos", [BL, N, 1], F32, kind="ExternalInput")
    const_specs = {
        "anch8": ([8, R], FP16),
        "ones16": ([2, N], FP16),
        "cb": ([128, 2048], BF16),
        "crow": ([1, 2 * D + 128], BF16),
        "cg": ([128, 1], F32),
        "caff": ([128, 2 * D], F32),
    }
    cd = {k: nc.dram_tensor(k, sh, dt, kind="ExternalInput")
          for k, (sh, dt) in const_specs.items()}
    out_d = nc.dram_tensor("out", [BL, N, D], BF16, kind="ExternalOutput")
    scratch_d = nc.dram_tensor("scratch", [BL, 4, N], FP16, kind="Internal")

    with tile.TileContext(nc) as tc:
        with tc.tile_pool(name="consts", bufs=1) as cp, \
             tc.tile_pool(name="emb", bufs=2) as embp, \
             tc.tile_pool(name="phi", bufs=2) as phip, \
             tc.tile_pool(name="coef", bufs=2) as coefp, \
             tc.tile_pool(name="pre", bufs=2) as prep, \
             tc.tile_pool(name="work", bufs=3) as wp, \
             tc.tile_pool(name="tiny", bufs=8) as tp, \
             tc.tile_pool(name="psB", bufs=1, space="PSUM") as psB:

            # ---- constants ----
            blob = {}
            for k, (sh, dt) in const_specs.items():
                if k == "caff" and not flags["ln2_aff"]:
                    continue
                blob[k] = cp.tile(sh, dt, tag=k, name=f"c_{k}")
                nc.sync.dma_start(blob[k][:], cd[k][tuple(slice(None) for _ in sh)])
            _cb = blob["cb"]
            ct = {
                "anch8": blob["anch8"],
                "qsub": _cb[:, 0:256],
                "qtw": _cb[:, 256:512].rearrange("p (a b) -> p a b", a=2),
                "slt": _cb[:, 512:640], "wq": _cb[:, 640:768],
                "mqt": _cb[:, 768:896],
                "wi": _cb[:, 896:1408].rearrange("p (a b) -> p a b", a=2),
                "wo": _cb[:, 1408:1920].rearrange("p (a b) -> p a b", a=2),
                "ident": _cb[:, 1920:2048],
                "bint_row": blob["crow"][:, 0:D],
                "brow": blob["crow"][:, D:2 * D],
                "ones1": blob["crow"][:, 2 * D:2 * D + 128],
                "epsb": blob["cg"][:, 0:1],
            }
            if flags["ln2_aff"]:
                ct["g2"] = blob["caff"][:, 0:D]
                ct["b2"] = blob["caff"][:, D:2 * D]

            import contextlib
            loopctx = tc.For_i(0, repeats, 1) if repeats > 1 else contextlib.nullcontext()
            with loopctx:
              st = [dict() for _ in range(BL)]

              def load_emb(b):
                  s = st[b]
                  s["emb"] = embp.tile([128, NT, D], BF16, tag="emb",
                                       name=f"emb_{b}")
                  eap = emb_d[b].rearrange("(t q) d -> q t d", q=128)
                  for k4 in range(4):
                      nc.sync.dma_start(s["emb"][:, 8 * k4:8 * (k4 + 1), :],
                                        eap[:, 8 * k4:8 * (k4 + 1), :])

              def prologue(b):
                  """fp16 split rows: scratch rows [ph, pl, qh, qlr]."""
                  s = st[b]
                  p16 = prep.tile([16, 256], F32, tag="p16", name=f"p16_{b}")
                  nc.sync.dma_start(p16[:],
                                    pos_d[b, :, 0].rearrange("(k j) -> k j", k=16))
                  ph = prep.tile([16, 256], FP16, tag="ph", name=f"ph_{b}")
                  nc.vector.tensor_copy(ph[:], p16[:])
                  dd = prep.tile([16, 256], F32, tag="dd", name=f"dd_{b}")
                  nc.vector.tensor_sub(dd[:], p16[:], ph[:])
                  pl = prep.tile([16, 256], FP16, tag="pl", name=f"pl_{b}")
                  nc.vector.tensor_scalar(pl[:], dd[:], 4096.0, None, op0=ALU.mult)
                  qq = prep.tile([16, 256], F32, tag="qq", name=f"qq_{b}")
                  nc.vector.tensor_mul(qq[:], ph[:], ph[:])
                  qh = prep.tile([16, 256], FP16, tag="qh", name=f"qh_{b}")
                  nc.vector.tensor_copy(qh[:], qq[:])
                  ee = prep.tile([16, 256], F32, tag="ee", name=f"ee_{b}")
                  nc.vector.tensor_sub(ee[:], qq[:], qh[:])
                  rr = prep.tile([16, 256], F32, tag="rr", name=f"rr_{b}")
                  nc.vector.tensor_mul(rr[:], ph[:], pl[:])
                  qlr = prep.tile([16, 256], FP16, tag="qlr", name=f"qlr_{b}")
                  nc.vector.scalar_tensor_tensor(qlr[:], ee[:], 2048.0, rr[:],
                                                 op0=ALU.mult, op1=ALU.add)
                  from concourse.tile_rust import add_dep_helper
                  iw = []
                  for i, t in enumerate((ph, pl, qh, qlr)):
                      iw.append(nc.sync.dma_start(
                          scratch_d[b, i].rearrange("(k j) -> k j", k=16), t[:]))
                  pp8 = prep.tile([8, N], FP16, tag="pp8", name=f"pp8_{b}")
                  rd = [(0, 2), (1, 2), (2, 3), (3, 0), (4, 0), (5, 1)]
                  for row, src in rd:
                      ir = nc.sync.dma_start(
                          pp8[row:row + 1, :],
                          scratch_d[b, src].rearrange("(one n) -> one n", one=1))
                      add_dep_helper(ir.ins, iw[src].ins, sync=True,
                                     reason="scratch RAW")
                  nc.sync.dma_start(pp8[6:8, :], cd["ones16"][:, :])
                  s["pp8"] = pp8

              def stage1_init(b):
                  s = st[b]
                  phiT = phip.tile([R, 8, 512], BF16, tag="phiT", name=f"phiT_{b}")
                  phiN = phip.tile([128, NT, 128], BF16, tag="phiN",
                                   name=f"phiN_{b}")
                  s["phiT"], s["phiN"] = phiT, phiN
                  s["pCt"] = psB.tile([128, 2, 256], F32, tag="ps2", bufs=6,
                                      name=f"pC_{b}")

              def stage1_chunk(b, j):
                  s = st[b]
                  pp8, emb_sb = s["pp8"], s["emb"]
                  phiT, phiN = s["phiT"], s["phiN"]
                  pC = s["pCt"][:, 0, :]
                  if True:
                      psPhi = psB.tile([128, 2, 256], F32, tag="ps2", bufs=6,
                                       name=f"psPhi_{b}_{j}")
                      psPhiv = psPhi[:].rearrange("p a b -> p (a b)")
                      nc.tensor.matmul(psPhiv, ct["anch8"][:, :],
                                       pp8[:, 512 * j:512 * (j + 1)],
                                       start=True, stop=True)
                      nc.scalar.activation(phiT[:, j, :], psPhiv, ACTF.Exp)
                      ptT = psB.tile([128, 512], BF16, tag="psbf", bufs=2,
                                     name=f"ptT_{b}_{j}")
                      for h in range(4):
                          nc.tensor.transpose(ptT[:, 128 * h:128 * (h + 1)],
                                              phiT[:, j, 128 * h:128 * (h + 1)],
                                              ct["ident"][:, :])
                      # evac: alternate DVE / Act (Pool cannot read PSUM)
                      dst = phiN[:, 4 * j:4 * (j + 1), :].rearrange("p a b -> p (a b)")
                      if j % 2 == 0:
                          nc.vector.tensor_copy(dst, ptT[:])
                      else:
                          nc.scalar.copy(dst, ptT[:])
                      for h in range(4):
                          t = 4 * j + h
                          nc.tensor.matmul(pC, phiN[:, t, :], emb_sb[:, t, :],
                                           start=(t == 0), stop=(t == NT - 1))

              def stage1_fin(b):
                  s = st[b]
                  pC = s["pCt"][:, 0, :]
                  craw = coefp.tile([R, D], BF16, tag="craw", name=f"craw_{b}")
                  nc.scalar.copy(craw[:], pC)
                  pC2t = psB.tile([128, 2, 256], F32, tag="ps2", bufs=6,
                                  name=f"pC2_{b}")
                  pC2 = pC2t[:, 0, :]
                  nc.tensor.matmul(pC2, ct["wq"][:, :], craw[:],
                                   start=True, stop=True)
                  C = coefp.tile([R, D], BF16, tag="C", bufs=4, name=f"C_{b}")
                  nc.vector.tensor_copy(C[:], pC2)
                  s["C"] = C

              def diffuse_step(b, step):
                  s = st[b]
                  C = s["C"]
                  ptC = psB.tile([128, 512], BF16, tag="psbf", bufs=2,
                                 name=f"ptC_{b}_{step}")
                  for h in range(2):
                      nc.tensor.transpose(ptC[:, 128 * h:128 * (h + 1)],
                                          C[:, 128 * h:128 * (h + 1)],
                                          ct["ident"][:, :])
                  Ct = wp.tile([128, 2, 128], BF16, tag="Ct", name=f"Ct_{b}_{step}")
                  nc.vector.tensor_copy(
                      Ct[:].rearrange("p a b -> p (a b)"), ptC[:, 0:256])
                  pCWt = psB.tile([128, 2, 256], F32, tag="ps2", bufs=6,
                                  name=f"pCW_{b}_{step}")
                  pCW = pCWt[:, 0, :]
                  for h in range(2):
                      nc.tensor.matmul(pCW, Ct[:, h, :], ct["wi"][:, h, :],
                                       start=(h == 0), stop=(h == 1))
                  CWb = wp.tile([R, D], BF16, tag="CWb", name=f"CWb_{b}_{step}")
                  nc.scalar.copy(CWb[:], pCW)
                  psF = psB.tile([128, 2, 256], F32, tag="ps2", bufs=6,
                                 name=f"psF_{b}_{step}")
                  for sc in range(2):
                      nc.tensor.matmul(psF[:, sc, :],
                                       ct["qsub"][:, 128 * sc:128 * (sc + 1)],
                                       CWb[:], start=True,
                                       stop=not flags["use_bint"])
                      if flags["use_bint"]:
                          nc.tensor.matmul(psF[:, sc, :], ct["ones1"][0:1, :],
                                           ct["bint_row"][0:1, :],
                                           start=False, stop=True)
                  T = wp.tile([128, 2, 256], BF16, tag="T", name=f"T_{b}_{step}")
                  nc.scalar.activation(T[:].rearrange("p a b -> p (a b)"),
                                       psF[:].rearrange("p a b -> p (a b)"),
                                       ACTF.Tanh)
                  pCnt = psB.tile([128, 2, 256], F32, tag="ps2", bufs=6,
                                  name=f"pCn_{b}_{step}")
                  pCn = pCnt[:, 0, :]
                  nc.tensor.matmul(pCn, ct["slt"][:, :], C[:],
                                   start=True, stop=False)
                  for sc in range(2):
                      nc.tensor.matmul(pCn, ct["qtw"][:, sc, :], T[:, sc, :],
                                       start=False, stop=(sc == 1))
                  C2 = coefp.tile([R, D], BF16, tag="C", bufs=4,
                                  name=f"C_{b}_{step}")
                  nc.vector.tensor_copy(C2[:], pCn)
                  s["C"] = C2

              def finish_coef(b):
                  s = st[b]
                  pMCt = psB.tile([128, 2, 256], F32, tag="ps2", bufs=6,
                                  name=f"pMC_{b}")
                  pMC = pMCt[:, 0, :]
                  nc.tensor.matmul(pMC, ct["mqt"][:, :], s["C"][:],
                                   start=True, stop=True)
                  MC = coefp.tile([R, D], BF16, tag="MC", name=f"MC_{b}")
                  nc.scalar.copy(MC[:], pMC)
                  s["MC"] = MC

              def ep_a(b, p):
                  """psamp matmuls -> psX"""
                  s = st[b]
                  phiT, MC, emb_sb = s["phiT"], s["MC"], s["emb"]
                  e = s.setdefault("ep", {}).setdefault(p, {})
                  psX = psB.tile([128, 2, 256], F32, tag="ps2", bufs=6,
                                 name=f"psX_{b}_{p}")
                  e["psX"] = psX
                  for tp in range(2):
                      t = 2 * p + tp
                      jc, h = divmod(t, 4)
                      embadd = (not SAFE) and (tp == 1)
                      nc.tensor.matmul(psX[:, tp, :],
                                       phiT[:, jc, 128 * h:128 * (h + 1)],
                                       MC[:], start=True, stop=not embadd)
                      if embadd:
                          nc.tensor.matmul(psX[:, tp, :], ct["ident"][:, :],
                                           emb_sb[:, t, :], start=False,
                                           stop=True)

              def ep_b(b, p):
                  """x evac + row sums"""
                  s = st[b]
                  emb_sb = s["emb"]
                  e = s["ep"][p]
                  psX = e["psX"]
                  x_bf = wp.tile([128, 2, 256], BF16, tag="x", bufs=6,
                                 name=f"x_{b}_{p}")
                  sx = tp_.tile([128, 4], F32, tag="sx", name=f"sx_{b}_{p}")
                  e["x_bf"], e["sx"] = x_bf, sx
                  if SAFE:
                      for tp in range(2):
                          t = 2 * p + tp
                          nc.vector.scalar_tensor_tensor(
                              x_bf[:, tp, :], psX[:, tp, :], 1.0,
                              emb_sb[:, t, :], op0=ALU.mult, op1=ALU.add,
                              accum_out=sx[:, tp:tp + 1])
                      return
                  t0 = 2 * p
                  nc.vector.scalar_tensor_tensor(x_bf[:, 0, :], psX[:, 0, :],
                                                 1.0, emb_sb[:, t0, :],
                                                 op0=ALU.mult, op1=ALU.add,
                                                 accum_out=sx[:, 0:1])
                  nc.scalar.activation(x_bf[:, 1, :], psX[:, 1, :],
                                       ACTF.Identity, accum_out=sx[:, 1:2])

              def ep_c(b, p):
                  """LN1 stats"""
                  s = st[b]
                  e = s["ep"][p]
                  x_bf, sx = e["x_bf"], e["sx"]
                  invD = 1.0 / D
                  mv = tp_.tile([128, 4], F32, tag="mv", name=f"mv_{b}_{p}")
                  rst = tp_.tile([128, 2], F32, tag="rst", name=f"rst_{b}_{p}")
                  e["mv"], e["rst"] = mv, rst
                  junk = wp.tile([128, 2, 256], BF16, tag="junk", bufs=3,
                                 name=f"junk_{b}_{p}")
                  for tp in range(2):
                      nc.vector.affine_mul_reduce(
                          junk[:, tp, :], sx[:, 2 + tp:3 + tp],
                          x_bf[:, tp, :], x_bf[:, tp, :], 1.0, 0.0)
                  for tp in range(2):
                      nc.vector.tensor_scalar(mv[:, tp:tp + 1], sx[:, tp:tp + 1],
                                              invD, None, op0=ALU.mult)
                      nc.vector.tensor_mul(mv[:, 2 + tp:3 + tp],
                                           mv[:, tp:tp + 1], mv[:, tp:tp + 1])
                      nc.vector.scalar_tensor_tensor(
                          mv[:, 2 + tp:3 + tp], sx[:, 2 + tp:3 + tp], invD,
                          mv[:, 2 + tp:3 + tp], op0=ALU.mult, op1=ALU.subtract)
                      nc.scalar.activation(rst[:, tp:tp + 1],
                                           mv[:, 2 + tp:3 + tp],
                                           ACTF.Sqrt, bias=ct["epsb"][:, :])
                  nc.vector.reciprocal(rst[:], rst[:])

              def ep_d(b, p):
                  """normalize, transpose, v matmul -> psV"""
                  s = st[b]
                  e = s["ep"][p]
                  x_bf, mv, rst = e["x_bf"], e["mv"], e["rst"]
                  enh = wp.tile([128, 2, 256], BF16, tag="enh", bufs=4,
                                name=f"enh_{b}_{p}")
                  ptE = psB.tile([128, 512], BF16, tag="psbf", bufs=2,
                                 name=f"ptE_{b}_{p}")
                  for tp in range(2):
                      nc.vector.tensor_scalar(enh[:, tp, :], x_bf[:, tp, :],
                                              mv[:, tp:tp + 1], rst[:, tp:tp + 1],
                                              op0=ALU.subtract, op1=ALU.mult)
                      for h2 in range(2):
                          nc.tensor.transpose(
                              ptE[:, 256 * tp + 128 * h2:256 * tp + 128 * (h2 + 1)],
                              enh[:, tp, 128 * h2:128 * (h2 + 1)],
                              ct["ident"][:, :])
                  enhT = wp.tile([128, 4, 128], BF16, tag="enhT", bufs=4,
                                 name=f"enhT_{b}_{p}")
                  if (not SAFE) and p % 2 == 1:
                      nc.scalar.copy(enhT[:].rearrange("p a b -> p (a b)"), ptE[:])
                  else:
                      nc.vector.tensor_copy(enhT[:].rearrange("p a b -> p (a b)"),
                                            ptE[:])
                  psV = psB.tile([128, 2, 256], F32, tag="ps2", bufs=6,
                                 name=f"psV_{b}_{p}")
                  e["psV"] = psV
                  for tp in range(2):
                      for h2 in range(2):
                          nc.tensor.matmul(psV[:, tp, :], enhT[:, 2 * tp + h2, :],
                                           ct["wo"][:, h2, :],
                                           start=(h2 == 0),
                                           stop=(h2 == 1 and not flags["use_brow"]))
                      if flags["use_brow"]:
                          nc.tensor.matmul(psV[:, tp, :], ct["ones1"][0:1, :],
                                           ct["brow"][0:1, :],
                                           start=False, stop=True)

              def ep_e(b, p):
                  """v evac + row sums"""
                  s = st[b]
                  e = s["ep"][p]
                  psV = e["psV"]
                  v_bf = wp.tile([128, 2, 256], BF16, tag="v", bufs=6,
                                 name=f"v_{b}_{p}")
                  sv = tp_.tile([128, 4], F32, tag="sv", name=f"sv_{b}_{p}")
                  e["v_bf"], e["sv"] = v_bf, sv
                  if SAFE:
                      for tp in range(2):
                          nc.vector.tensor_scalar(v_bf[:, tp, :], psV[:, tp, :],
                                                  1.0, 0.0, op0=ALU.mult,
                                                  op1=ALU.add,
                                                  accum_out=sv[:, tp:tp + 1])
                      return
                  nc.scalar.activation(v_bf[:, 0, :], psV[:, 0, :],
                                       ACTF.Identity, accum_out=sv[:, 0:1])
                  nc.vector.tensor_scalar(v_bf[:, 1, :], psV[:, 1, :],
                                          1.0, 0.0, op0=ALU.mult, op1=ALU.add,
                                          accum_out=sv[:, 1:2])

              def ep_f(b, p):
                  """LN2 stats"""
                  s = st[b]
                  e = s["ep"][p]
                  v_bf, sv = e["v_bf"], e["sv"]
                  invD = 1.0 / D
                  mv2 = tp_.tile([128, 4], F32, tag="mv2", name=f"mv2_{b}_{p}")
                  rst2 = tp_.tile([128, 2], F32, tag="rst2", name=f"rst2_{b}_{p}")
                  e["mv2"], e["rst2"] = mv2, rst2
                  junk2 = wp.tile([128, 2, 256], BF16, tag="junk2", bufs=3,
                                  name=f"junk2_{b}_{p}")
                  for tp in range(2):
                      nc.vector.affine_mul_reduce(
                          junk2[:, tp, :], sv[:, 2 + tp:3 + tp],
                          v_bf[:, tp, :], v_bf[:, tp, :], 1.0, 0.0)
                  for tp in range(2):
                      nc.vector.tensor_scalar(mv2[:, tp:tp + 1], sv[:, tp:tp + 1],
                                              invD, None, op0=ALU.mult)
                      nc.vector.tensor_mul(mv2[:, 2 + tp:3 + tp],
                                           mv2[:, tp:tp + 1], mv2[:, tp:tp + 1])
                      nc.vector.scalar_tensor_tensor(
                          mv2[:, 2 + tp:3 + tp], sv[:, 2 + tp:3 + tp], invD,
                          mv2[:, 2 + tp:3 + tp], op0=ALU.mult, op1=ALU.subtract)
                      nc.scalar.activation(rst2[:, tp:tp + 1],
                                           mv2[:, 2 + tp:3 + tp],
                                           ACTF.Sqrt, bias=ct["epsb"][:, :])
                  nc.vector.reciprocal(rst2[:], rst2[:])

              def ep_g(b, p):
                  """final normalize + DMA out"""
                  s = st[b]
                  e = s["ep"][p]
                  v_bf, mv2, rst2 = e["v_bf"], e["mv2"], e["rst2"]
                  ot = wp.tile([128, 2, 256], BF16, tag="ot", bufs=4,
                               name=f"ot_{b}_{p}")
                  for tp in range(2):
                      nc.vector.tensor_scalar(ot[:, tp, :], v_bf[:, tp, :],
                                              mv2[:, tp:tp + 1],
                                              rst2[:, tp:tp + 1],
                                              op0=ALU.subtract, op1=ALU.mult)
                      if flags["ln2_aff"]:
                          nc.vector.tensor_mul(ot[:, tp, :], ot[:, tp, :],
                                               ct["g2"][:, :])
                          nc.vector.tensor_add(ot[:, tp, :], ot[:, tp, :],
                                               ct["b2"][:, :])
                  nc.sync.dma_start(
                      out_d[b].rearrange("(t q) d -> q t d", q=128)
                           [:, 2 * p:2 * (p + 1), :], ot[:])
                  s["ep"].pop(p)

              tp_ = tp
              # ---- emission: interleave the two batches at fine grain ----
              for b in range(BL):
                  prologue(b)
              for b in range(BL):
                  load_emb(b)
              if "s1" in parts:
                  for b in range(BL):
                      stage1_init(b)
                  for j in range(8):
                      for b in range(BL):
                          stage1_chunk(b, j)
                  for b in range(BL):
                      stage1_fin(b)
                  if "diff" in parts:
                      for step in range(NUM_STEPS):
                          for b in range(BL):
                              diffuse_step(b, step)
                  for b in range(BL):
                      finish_coef(b)
                  if "epi" in parts:
                      units = [(b, p) for p in range(NT // 2)
                               for b in range(BL)]
                      GP = 3
                      for g0 in range(0, len(units), GP):
                          grp = units[g0:g0 + GP]
                          for ph in (ep_a, ep_b, ep_c, ep_d, ep_e, ep_f, ep_g):
                              for (b, p) in grp:
                                  ph(b, p)

    nc.compile()
    return nc


# --------------------------------------------------------------------------
# runner (same multi-core pjrt path as before)
# --------------------------------------------------------------------------
def _make_runner(nc):
    import jax
    import numpy as _np
    from jax.sharding import Mesh, PartitionSpec
    from jax.experimental.shard_map import shard_map
    from concourse import mybir as _mb
    from concourse.bass2jax import (install_neuronx_cc_hook, _bass_exec_p,
                                    partition_id_tensor)
    install_neuronx_cc_hook()
    partition_name = nc.partition_id_tensor.name if nc.partition_id_tensor else None
    in_names, out_names, out_avals, zero_outs = [], [], [], []
    for alloc in nc.m.functions[0].allocations:
        if not isinstance(alloc, _mb.MemoryLocationSet):
            continue
        name = alloc.memorylocations[0].name
        if alloc.kind == "ExternalInput":
            if name != partition_name:
                in_names.append(name)
        elif alloc.kind == "ExternalOutput":
            npdt = _mb.dt.np(alloc.dtype)
            out_names.append(name)
            out_avals.append(jax.core.ShapedArray(tuple(alloc.tensor_shape), npdt))
            zero_outs.append(_np.zeros(tuple(alloc.tensor_shape), npdt))
    n_params = len(in_names)
    n_outs = len(out_names)
    all_in = in_names + out_names + ([partition_name] if partition_name else [])

    def _body(*args):
        operands = list(args)
        if partition_name is not None:
            operands.append(partition_id_tensor())
        return tuple(_bass_exec_p.bind(
            *operands, out_avals=tuple(out_avals),
            in_names=tuple(all_in), out_names=tuple(out_names),
            lowering_input_output_aliases=(), sim_require_finite=True,
            sim_require_nnan=True, nc=nc))

    devices = jax.devices()[:NCORES]
    mesh = Mesh(_np.asarray(devices), ("core",))
    donate = tuple(range(n_params, n_params + n_outs))
    sharded = jax.jit(
        shard_map(_body, mesh=mesh,
                  in_specs=(PartitionSpec("core"),) * (n_params + n_outs),
                  out_specs=(PartitionSpec("core"),) * n_outs,
                  check_rep=False),
        donate_argnums=donate, keep_unused=True)

    def run(in_maps):
        per_core = [[_np.asarray(m[name]) for name in in_names] for m in in_maps]
        concat_in = [_np.concatenate([per_core[c][i] for c in range(NCORES)], axis=0)
                     for i in range(n_params)]
        concat_zero = [_np.zeros((NCORES * z.shape[0], *z.shape[1:]), z.dtype)
                       for z in zero_outs]
        outs = sharded(*concat_in, *concat_zero)
        outs = [_np.asarray(o) for o in outs]
        return {name: outs[i] for i, name in enumerate(out_names)}

    return run


def kernel(**inputs):
    import ml_dtypes
    emb = np.ascontiguousarray(inputs["embeddings"], dtype=np.float32)
    pos = np.ascontiguousarray(inputs["positions"], dtype=np.float32)
    grid = np.asarray(inputs["grid_points"], np.float64)[0, :, 0]
    params = dict(
        sigma=float(np.asarray(inputs["sigma"])),
        alpha=float(np.asarray(inputs["alpha"])),
        grid=grid,
        W_int=np.asarray(inputs["W_int"], np.float64),
        b_int=np.asarray(inputs["b_int"], np.float64),
        W_out=np.asarray(inputs["W_out"], np.float64),
        b_out=np.asarray(inputs["b_out"], np.float64),
        ln1_g=np.asarray(inputs["ln1_g"], np.float64),
        ln1_b=np.asarray(inputs["ln1_b"], np.float64),
        ln2_g=np.asarray(inputs["ln2_g"], np.float64),
        ln2_b=np.asarray(inputs["ln2_b"], np.float64),
    )
    key = hashlib.sha256(b"".join(np.asarray(v).tobytes() for v in params.values())).hexdigest()
    if key not in _CACHE:
        consts, flags = _host_plan(**params)
        nc = _build_module(flags)
        _CACHE[key] = (_make_runner(nc), consts)
    run, consts = _CACHE[key]

    embb = emb.astype(ml_dtypes.bfloat16)
    in_maps = []
    for c in range(NCORES):
        m = {"emb": embb[BL * c:BL * (c + 1)],
             "pos": pos[BL * c:BL * (c + 1)]}
        m.update(consts)
        in_maps.append(m)
    outs = run(in_maps)
    return np.ascontiguousarray(outs["out"].astype(np.float32))


# revision 22
# speedup vs baseline: 1.7902x; 1.2789x over previous
"""Trainium2 Bass kernel for nn_EnhancedTFNLayer (RBF field projection +
diffusion + sampling + LN/linear epilogue), data-parallel over batch on 8 cores.

Low-rank field pipeline (R=128 orthonormal basis Q fitted on host from the
parameter inputs only):

  phi[n, j] = exp(-(p_n - c_j)^2 / (2 s^2))   anchor features (fp16
              split-precision K=8 matmul + Exp)
  C = Wq^T (phi^T emb)                        field coords
  4x diffusion: C' = SLQ C + QTW @ tanh(Qsub^T (C W_int) + b_int)
              (tanh evaluated on a 256-point subsampled grid; QTW is a
               host-fitted quadrature back-projection, factor DT included)
  sampled = phi (MQ C)
  x = sampled + emb ; out = LN2(LN1(x) @ (W_out + I))   [residuals folded]

All matmul operands bf16 (emb converted on host); LN stats via stt/ts
accum_out (sum) + tensor_tensor_reduce (sum of squares); PSUM evacuations
spread across DVE / Pool / Act engines.
"""
import sys
import hashlib
import numpy as np

for _p in ("/opt/trn_rl_repo", "/root/.axon_site/_ro/trn_rl_repo"):
    if _p not in sys.path:
        sys.path.insert(0, _p)

import concourse.bass as bass
import concourse.bacc as bacc
import concourse.tile as tile
from concourse import mybir

F32 = mybir.dt.float32
BF16 = mybir.dt.bfloat16
FP16 = mybir.dt.float16
ACTF = mybir.ActivationFunctionType
ALU = mybir.AluOpType
AXL = mybir.AxisListType

B, N, G, D = 16, 4096, 1024, 256
NUM_STEPS, DT, EPS = 4, 0.01, 1e-5
R = 128
SSUB = 256               # tanh-subsampled grid points
NT = N // 128            # 32 token tiles per batch
BL = 2                   # batches per core
NCORES = 8

_CACHE = {}


def _bf16(x):
    x = np.ascontiguousarray(x, np.float32)
    u = x.view(np.uint32)
    r = ((u >> 16) + ((u >> 15) & 1)).astype(np.uint32) << 16
    return r.view(np.float32)


def _fp16(x):
    return np.float16(np.asarray(x, np.float64).astype(np.float32)).astype(np.float32)


# --------------------------------------------------------------------------
# host-side operator fitting (float64; parameter inputs only)
# --------------------------------------------------------------------------
def _host_plan(sigma, alpha, grid, W_int, b_int, W_out, b_out,
               ln1_g, ln1_b, ln2_g, ln2_b):
    rng = np.random.default_rng(0)
    c0 = 1.0 - 2.0 * alpha * DT
    c1 = alpha * DT
    pg = np.linspace(0.0, 1.0, 8193)
    K = np.exp(-((pg[:, None] - grid[None, :]) ** 2) / (2 * sigma * sigma))
    nsyn = 384
    sub = rng.choice(len(pg), size=256, replace=False)
    Fsyn = K[sub].T @ rng.standard_normal((256, nsyn))
    Fsyn /= np.abs(Fsyn).max(0, keepdims=True) + 1e-30
    fscale = np.sqrt(N * sigma * np.sqrt(np.pi))
    wnorm = np.linalg.norm(W_int, axis=0)
    wcols = rng.choice(len(wnorm), size=nsyn)
    gains = fscale * wnorm[wcols] * rng.uniform(0.5, 2.0, nsyn)
    Tsyn = np.tanh(Fsyn * gains[None, :])
    Msvd = np.concatenate([K, (Tsyn * 0.1).T], axis=0)
    _, _, Vt = np.linalg.svd(Msvd, full_matrices=False)
    Q = Vt[:R]                                            # [R, G] orthonormal
    # anchors
    c = np.linspace(-0.08, 1.08, R)
    s = 2.2 * (c[1] - c[0])
    F = np.exp(-((pg[:, None] - c[None, :]) ** 2) / (2 * s * s))
    Qk = K @ Q.T
    Wq, *_ = np.linalg.lstsq(F, Qk, rcond=1e-8)           # [R, R]
    Qt = Q.T
    LQt = c0 * Qt.copy()
    LQt[1:-1] += c1 * (Qt[:-2] + Qt[2:])
    LQt[0] += c1 * (Qt[0] + Qt[1])
    LQt[-1] += c1 * (Qt[-2] + Qt[-1])
    SLQ = Q @ LQt                                         # [R, R]
    u = pg * (G - 1)
    i0 = np.clip(np.floor(u), 0, G - 2).astype(int)
    w = u - i0
    lerpQ = Qt[i0] * (1 - w)[:, None] + Qt[i0 + 1] * w[:, None]
    MQ, *_ = np.linalg.lstsq(F, lerpQ, rcond=1e-5)        # [R, R]

    # subsampled-tanh quadrature back-projection QTW [R, SSUB]
    subidx = np.unique(np.linspace(0, G - 1, SSUB).round().astype(int))
    assert len(subidx) == SSUB
    nsyn2 = 1024
    Fg = np.exp(-((grid[:, None] - grid[None, ::8]) ** 2) / (2 * sigma * sigma))
    fields = Fg @ rng.standard_normal((Fg.shape[1], nsyn2))
    fields /= np.abs(fields).max(0, keepdims=True) + 1e-30
    gains2 = fscale * wnorm[rng.choice(len(wnorm), size=nsyn2)] * \
        np.exp(rng.uniform(np.log(0.25), np.log(4.0), nsyn2))
    TG = np.tanh(fields * gains2[None, :])                # [G, nsyn2]
    target = Q @ TG
    A = TG[subidx, :]
    lam = 1e-6 * np.linalg.norm(A) ** 2 / A.shape[0]
    QTW = np.linalg.solve(A @ A.T + lam * np.eye(SSUB), A @ target.T).T

    # fp16 split-precision anchor coefficient matrix [8, R]
    # pp8 rows on device: [qh, qh, qlr, ph, ph, pl, 1, 1]
    a3 = -1.0 / (2 * s * s)
    a1 = c / (s * s)
    a2 = -c * c / (2 * s * s)
    a3h = _fp16(a3); a3l = a3 - a3h
    a1h = _fp16(a1); a1l = a1 - a1h
    a2h = _fp16(a2); a2l = a2 - a2h
    anch8 = np.stack([
        np.full(R, a3h), np.full(R, a3l), np.full(R, a3 / 2048.0),
        a1h, a1l, a1 / 4096.0,
        a2h, a2l,
    ], axis=0)

    # affine folds: enh_aff = enh*g1 + b1 ; v = enh_aff @ (W_out + I) + b_out
    Wp = ln1_g[:, None] * (W_out + np.eye(D))             # rows scaled by g1
    brow = b_out + ln1_b @ (W_out + np.eye(D))            # const row
    f32 = lambda x: np.ascontiguousarray(x, dtype=np.float32)
    f16 = lambda x: np.ascontiguousarray(x, dtype=np.float16)

    # bf16 const blob [128, W] (values pre-rounded to bf16, stored as f32 on
    # host; device tile dtype BF16 so DMA converts? no -- DMA does not convert.
    # Host passes ml_dtypes.bfloat16 array instead; see _pack_bf16.)
    qsub = Q[:, subidx]                                   # [R, SSUB]
    qtw_t = (QTW * DT).T.reshape(2, 128, R).transpose(1, 0, 2)  # [128,2,R]
    wi = W_int.reshape(2, 128, D).transpose(1, 0, 2)      # [128,2,D]
    wo = Wp.reshape(2, 128, D).transpose(1, 0, 2)         # [128,2,D]
    cb = np.concatenate([
        qsub,                                             # [:,0:256]
        qtw_t.reshape(128, 2 * R),                        # [:,256:512]
        SLQ.T, Wq, MQ.T,                                  # 512:640,640:768,768:896
        wi.reshape(128, 2 * D),                           # 896:1408
        wo.reshape(128, 2 * D),                           # 1408:1920
        np.eye(128),                                      # 1920:2048
    ], axis=1)
    # row blob (bf16) [1, 512+]: bint row | brow | ones128
    crow = np.concatenate([
        b_int.reshape(1, D), brow.reshape(1, D), np.ones((1, 128)),
    ], axis=1)
    # f32 misc blob [128, 5]: epsb | g2? b2? (ln2 affine rows go separately)
    cg = np.full((128, 1), EPS)
    # ln2 affine rows [128, 2*D] f32 (only DMA'd/used when ln2_aff)
    caff = np.concatenate([np.broadcast_to(ln2_g, (128, D)),
                           np.broadcast_to(ln2_b, (128, D))], axis=1)

    import ml_dtypes
    bfl = lambda x: np.ascontiguousarray(x, dtype=ml_dtypes.bfloat16)
    consts = {
        "anch8": f16(anch8),
        "ones16": f16(np.ones((2, N))),
        "cb": bfl(cb),
        "crow": bfl(crow),
        "cg": f32(cg),
        "caff": f32(caff),
    }
    flags = {
        "use_bint": bool(np.any(b_int != 0)),
        "use_brow": bool(np.any(np.abs(brow) > 1e-12)),
        "ln2_aff": bool(np.any(ln2_g != 1) or np.any(ln2_b != 0)),
    }
    return consts, flags


# --------------------------------------------------------------------------
# device module
# --------------------------------------------------------------------------
def _build_module(flags, repeats=1, parts=("s1", "diff", "epi")):
    import os
    SAFE = os.environ.get("SAFE", "0") == "1"
    FEATS = set(os.environ.get("FEATS", "").split(","))
    nc = bacc.Bacc(trn_type="TRN2")
    emb_d = nc.dram_tensor("emb", [BL, N, D], BF16, kind="ExternalInput")
    pos_d = nc.dram_tensor("pos", [BL, N, 1], F32, kind="ExternalInput")
    const_specs = {
        "anch8": ([8, R], FP16),
        "ones16": ([2, N], FP16),
        "cb": ([128, 2048], BF16),
        "crow": ([1, 2 * D + 128], BF16),
        "cg": ([128, 1], F32),
        "caff": ([128, 2 * D], F32),
    }
    cd = {k: nc.dram_tensor(k, sh, dt, kind="ExternalInput")
          for k, (sh, dt) in const_specs.items()}
    out_d = nc.dram_tensor("out", [BL, N, D], BF16, kind="ExternalOutput")
    scratch_d = nc.dram_tensor("scratch", [BL, 8, N], FP16, kind="Internal")

    with tile.TileContext(nc) as tc:
        with tc.tile_pool(name="consts", bufs=1) as cp, \
             tc.tile_pool(name="emb", bufs=2) as embp, \
             tc.tile_pool(name="phi", bufs=2) as phip, \
             tc.tile_pool(name="coef", bufs=2) as coefp, \
             tc.tile_pool(name="pre", bufs=2) as prep, \
             tc.tile_pool(name="work", bufs=3) as wp, \
             tc.tile_pool(name="tiny", bufs=8) as tp, \
             tc.tile_pool(name="psB", bufs=1, space="PSUM") as psB:

            # ---- constants ----
            blob = {}
            for k, (sh, dt) in const_specs.items():
                if k == "caff" and not flags["ln2_aff"]:
                    continue
                blob[k] = cp.tile(sh, dt, tag=k, name=f"c_{k}")
                nc.sync.dma_start(blob[k][:], cd[k][tuple(slice(None) for _ in sh)])
            _cb = blob["cb"]
            ct = {
                "anch8": blob["anch8"],
                "qsub": _cb[:, 0:256],
                "qtw": _cb[:, 256:512].rearrange("p (a b) -> p a b", a=2),
                "slt": _cb[:, 512:640], "wq": _cb[:, 640:768],
                "mqt": _cb[:, 768:896],
                "wi": _cb[:, 896:1408].rearrange("p (a b) -> p a b", a=2),
                "wo": _cb[:, 1408:1920].rearrange("p (a b) -> p a b", a=2),
                "ident": _cb[:, 1920:2048],
                "bint_row": blob["crow"][:, 0:D],
                "brow": blob["crow"][:, D:2 * D],
                "ones1": blob["crow"][:, 2 * D:2 * D + 128],
                "epsb": blob["cg"][:, 0:1],
            }
            if flags["ln2_aff"]:
                ct["g2"] = blob["caff"][:, 0:D]
                ct["b2"] = blob["caff"][:, D:2 * D]

            import contextlib
            loopctx = tc.For_i(0, repeats, 1) if repeats > 1 else contextlib.nullcontext()
            with loopctx:
              st = [dict() for _ in range(BL)]

              def load_emb(b):
                  s = st[b]
                  s["emb"] = embp.tile([128, NT, D], BF16, tag="emb",
                                       name=f"emb_{b}")
                  eap = emb_d[b].rearrange("(t q) d -> q t d", q=128)
                  for k4 in range(16):
                      nc.sync.dma_start(s["emb"][:, 2 * k4:2 * (k4 + 1), :],
                                        eap[:, 2 * k4:2 * (k4 + 1), :])

              def prologue(b):
                  """fp16 split rows -> scratch[8,N] -> pp8 in 3 DMAs.
                  rows: [qh, qh, qlr, ph, ph, pl, 1, 1]"""
                  s = st[b]
                  p16 = prep.tile([16, 256], F32, tag="p16", name=f"p16_{b}")
                  nc.gpsimd.dma_start(p16[:],
                                      pos_d[b, :, 0].rearrange("(k j) -> k j", k=16))
                  rows = prep.tile([16, 8, 256], FP16, tag="rows", name=f"rows_{b}")
                  ph = rows[:, 3, :]
                  nc.vector.tensor_copy(ph, p16[:])
                  nc.vector.tensor_copy(rows[:, 4, :], ph)
                  dd = prep.tile([16, 256], F32, tag="dd", name=f"dd_{b}")
                  nc.vector.tensor_sub(dd[:], p16[:], ph)
                  nc.vector.tensor_scalar(rows[:, 5, :], dd[:], 4096.0, None,
                                          op0=ALU.mult)
                  qq = prep.tile([16, 256], F32, tag="qq", name=f"qq_{b}")
                  nc.vector.tensor_mul(qq[:], ph, ph)
                  qh = rows[:, 0, :]
                  nc.vector.tensor_copy(qh, qq[:])
                  nc.vector.tensor_copy(rows[:, 1, :], qh)
                  ee = prep.tile([16, 256], F32, tag="ee", name=f"ee_{b}")
                  nc.vector.tensor_sub(ee[:], qq[:], qh)
                  rr = prep.tile([16, 256], F32, tag="rr", name=f"rr_{b}")
                  nc.vector.tensor_mul(rr[:], ph, rows[:, 5, :])
                  nc.vector.scalar_tensor_tensor(rows[:, 2, :], ee[:], 2048.0,
                                                 rr[:], op0=ALU.mult, op1=ALU.add)
                  nc.vector.memset(rows[:, 6:8, :].rearrange("p a b -> p (a b)"),
                                   1.0)
                  from concourse.tile_rust import add_dep_helper
                  iw = nc.gpsimd.dma_start(
                      scratch_d[b].rearrange("r (k j) -> k r j", k=16), rows[:])
                  pp8 = prep.tile([8, N], FP16, tag="pp8", name=f"pp8_{b}")
                  ir = nc.gpsimd.dma_start(pp8[:], scratch_d[b])
                  add_dep_helper(ir.ins, iw.ins, sync=True, reason="scratch RAW")
                  s["pp8"] = pp8

              def stage1_init(b):
                  s = st[b]
                  phiT = phip.tile([R, 8, 512], BF16, tag="phiT", name=f"phiT_{b}")
                  phiN = phip.tile([128, NT, 128], BF16, tag="phiN",
                                   name=f"phiN_{b}")
                  s["phiT"], s["phiN"] = phiT, phiN
                  s["pCt"] = psB.tile([128, 2, 256], F32, tag="ps2", bufs=6,
                                      name=f"pC_{b}")

              def stage1_chunk(b, j):
                  s = st[b]
                  pp8, emb_sb = s["pp8"], s["emb"]
                  phiT, phiN = s["phiT"], s["phiN"]
                  pC = s["pCt"][:, 0, :]
                  if True:
                      psPhi = psB.tile([128, 2, 256], F32, tag="ps2", bufs=6,
                                       name=f"psPhi_{b}_{j}")
                      psPhiv = psPhi[:].rearrange("p a b -> p (a b)")
                      nc.tensor.matmul(psPhiv, ct["anch8"][:, :],
                                       pp8[:, 512 * j:512 * (j + 1)],
                                       start=True, stop=True)
                      nc.scalar.activation(phiT[:, j, :], psPhiv, ACTF.Exp)
                      ptT = psB.tile([128, 512], BF16, tag="psbf", bufs=2,
                                     name=f"ptT_{b}_{j}")
                      for h in range(4):
                          nc.tensor.transpose(ptT[:, 128 * h:128 * (h + 1)],
                                              phiT[:, j, 128 * h:128 * (h + 1)],
                                              ct["ident"][:, :])
                      # evac: alternate DVE / Act (Pool cannot read PSUM)
                      dst = phiN[:, 4 * j:4 * (j + 1), :].rearrange("p a b -> p (a b)")
                      if j % 2 == 0:
                          nc.vector.tensor_copy(dst, ptT[:])
                      else:
                          nc.scalar.copy(dst, ptT[:])
                      for h in range(4):
                          t = 4 * j + h
                          nc.tensor.matmul(pC, phiN[:, t, :], emb_sb[:, t, :],
                                           start=(t == 0), stop=(t == NT - 1))

              def stage1_fin(b):
                  s = st[b]
                  pC = s["pCt"][:, 0, :]
                  craw = coefp.tile([R, D], BF16, tag="craw", name=f"craw_{b}")
                  nc.scalar.copy(craw[:], pC)
                  pC2t = psB.tile([128, 2, 256], F32, tag="ps2", bufs=6,
                                  name=f"pC2_{b}")
                  pC2 = pC2t[:, 0, :]
                  nc.tensor.matmul(pC2, ct["wq"][:, :], craw[:],
                                   start=True, stop=True)
                  C = coefp.tile([R, D], BF16, tag="C", bufs=4, name=f"C_{b}")
                  nc.vector.tensor_copy(C[:], pC2)
                  s["C"] = C

              def diffuse_step(b, step):
                  s = st[b]
                  C = s["C"]
                  ptC = psB.tile([128, 512], BF16, tag="psbf", bufs=2,
                                 name=f"ptC_{b}_{step}")
                  for h in range(2):
                      nc.tensor.transpose(ptC[:, 128 * h:128 * (h + 1)],
                                          C[:, 128 * h:128 * (h + 1)],
                                          ct["ident"][:, :])
                  Ct = wp.tile([128, 2, 128], BF16, tag="Ct", name=f"Ct_{b}_{step}")
                  nc.vector.tensor_copy(
                      Ct[:].rearrange("p a b -> p (a b)"), ptC[:, 0:256])
                  pCWt = psB.tile([128, 2, 256], F32, tag="ps2", bufs=6,
                                  name=f"pCW_{b}_{step}")
                  pCW = pCWt[:, 0, :]
                  for h in range(2):
                      nc.tensor.matmul(pCW, Ct[:, h, :], ct["wi"][:, h, :],
                                       start=(h == 0), stop=(h == 1))
                  CWb = wp.tile([R, D], BF16, tag="CWb", name=f"CWb_{b}_{step}")
                  nc.scalar.copy(CWb[:], pCW)
                  psF = psB.tile([128, 2, 256], F32, tag="ps2", bufs=6,
                                 name=f"psF_{b}_{step}")
                  for sc in range(2):
                      nc.tensor.matmul(psF[:, sc, :],
                                       ct["qsub"][:, 128 * sc:128 * (sc + 1)],
                                       CWb[:], start=True,
                                       stop=not flags["use_bint"])
                      if flags["use_bint"]:
                          nc.tensor.matmul(psF[:, sc, :], ct["ones1"][0:1, :],
                                           ct["bint_row"][0:1, :],
                                           start=False, stop=True)
                  T = wp.tile([128, 2, 256], BF16, tag="T", name=f"T_{b}_{step}")
                  nc.scalar.activation(T[:].rearrange("p a b -> p (a b)"),
                                       psF[:].rearrange("p a b -> p (a b)"),
                                       ACTF.Tanh)
                  pCnt = psB.tile([128, 2, 256], F32, tag="ps2", bufs=6,
                                  name=f"pCn_{b}_{step}")
                  pCn = pCnt[:, 0, :]
                  nc.tensor.matmul(pCn, ct["slt"][:, :], C[:],
                                   start=True, stop=False)
                  for sc in range(2):
                      nc.tensor.matmul(pCn, ct["qtw"][:, sc, :], T[:, sc, :],
                                       start=False, stop=(sc == 1))
                  C2 = coefp.tile([R, D], BF16, tag="C", bufs=4,
                                  name=f"C_{b}_{step}")
                  nc.vector.tensor_copy(C2[:], pCn)
                  s["C"] = C2

              def finish_coef(b):
                  s = st[b]
                  pMCt = psB.tile([128, 2, 256], F32, tag="ps2", bufs=6,
                                  name=f"pMC_{b}")
                  pMC = pMCt[:, 0, :]
                  nc.tensor.matmul(pMC, ct["mqt"][:, :], s["C"][:],
                                   start=True, stop=True)
                  MC = coefp.tile([R, D], BF16, tag="MC", name=f"MC_{b}")
                  nc.scalar.copy(MC[:], pMC)
                  s["MC"] = MC

              def ep_a(b, p):
                  """psamp matmuls -> psX"""
                  s = st[b]
                  phiT, MC, emb_sb = s["phiT"], s["MC"], s["emb"]
                  e = s.setdefault("ep", {}).setdefault(p, {})
                  psX = psB.tile([128, 2, 256], F32, tag="ps2", bufs=6,
                                 name=f"psX_{b}_{p}")
                  e["psX"] = psX
                  for tp in range(2):
                      t = 2 * p + tp
                      jc, h = divmod(t, 4)
                      embadd = not SAFE
                      nc.tensor.matmul(psX[:, tp, :],
                                       phiT[:, jc, 128 * h:128 * (h + 1)],
                                       MC[:], start=True, stop=not embadd)
                      if embadd:
                          nc.tensor.matmul(psX[:, tp, :], ct["ident"][:, :],
                                           emb_sb[:, t, :], start=False,
                                           stop=True)

              def ep_b(b, p):
                  """x evac + row sums"""
                  s = st[b]
                  emb_sb = s["emb"]
                  e = s["ep"][p]
                  psX = e["psX"]
                  x_bf = wp.tile([128, 2, 256], BF16, tag="x", bufs=6,
                                 name=f"x_{b}_{p}")
                  sx = tp_.tile([128, 4], F32, tag="sx", name=f"sx_{b}_{p}")
                  e["x_bf"], e["sx"] = x_bf, sx
                  if SAFE:
                      for tp in range(2):
                          t = 2 * p + tp
                          nc.vector.scalar_tensor_tensor(
                              x_bf[:, tp, :], psX[:, tp, :], 1.0,
                              emb_sb[:, t, :], op0=ALU.mult, op1=ALU.add,
                              accum_out=sx[:, tp:tp + 1])
                      return
                  for tp in range(2):
                      nc.scalar.activation(x_bf[:, tp, :], psX[:, tp, :],
                                           ACTF.Identity,
                                           accum_out=sx[:, tp:tp + 1])

              def ep_c(b, p):
                  """LN1 stats"""
                  s = st[b]
                  e = s["ep"][p]
                  x_bf, sx = e["x_bf"], e["sx"]
                  invD = 1.0 / D
                  mv = tp_.tile([128, 4], F32, tag="mv", name=f"mv_{b}_{p}")
                  e["mv"] = mv
                  junk = wp.tile([128, 2, 256], BF16, tag="junk", bufs=3,
                                 name=f"junk_{b}_{p}")
                  for tp in range(2):
                      nc.vector.affine_mul_reduce(
                          junk[:, tp, :], sx[:, 2 + tp:3 + tp],
                          x_bf[:, tp, :], x_bf[:, tp, :], 1.0, 0.0)
                  nc.vector.tensor_scalar(mv[:, 0:2], sx[:, 0:2],
                                          invD, None, op0=ALU.mult)
                  nc.vector.tensor_mul(mv[:, 2:4], mv[:, 0:2], mv[:, 0:2])
                  rst = tp_.tile([128, 2], F32, tag="rst", name=f"rst_{b}_{p}")
                  e["rst1"] = rst
                  nc.vector.scalar_tensor_tensor(
                      mv[:, 2:4], sx[:, 2:4], invD,
                      mv[:, 2:4], op0=ALU.mult, op1=ALU.subtract)
                  nc.scalar.activation(rst[:, 0:2], mv[:, 2:4],
                                       ACTF.Sqrt, bias=ct["epsb"][:, :])
                  nc.vector.reciprocal(rst[:], rst[:])

              def ep_d(b, p):
                  """normalize, transpose, v matmul -> psV"""
                  s = st[b]
                  e = s["ep"][p]
                  x_bf, mv, rst = e["x_bf"], e["mv"], e["rst1"]
                  enh = wp.tile([128, 2, 256], BF16, tag="enh", bufs=4,
                                name=f"enh_{b}_{p}")
                  ptE = psB.tile([128, 512], BF16, tag="psbf", bufs=2,
                                 name=f"ptE_{b}_{p}")
                  for tp in range(2):
                      nc.gpsimd.tensor_scalar(enh[:, tp, :], x_bf[:, tp, :],
                                              mv[:, tp:tp + 1], rst[:, tp:tp + 1],
                                              op0=ALU.subtract, op1=ALU.mult)
                      for h2 in range(2):
                          nc.tensor.transpose(
                              ptE[:, 256 * tp + 128 * h2:256 * tp + 128 * (h2 + 1)],
                              enh[:, tp, 128 * h2:128 * (h2 + 1)],
                              ct["ident"][:, :])
                  enhT = wp.tile([128, 4, 128], BF16, tag="enhT", bufs=4,
                                 name=f"enhT_{b}_{p}")
                  if (not SAFE) and p % 2 == 1:
                      nc.scalar.copy(enhT[:].rearrange("p a b -> p (a b)"), ptE[:])
                  else:
                      nc.vector.tensor_copy(enhT[:].rearrange("p a b -> p (a b)"),
                                            ptE[:])
                  psV = psB.tile([128, 2, 256], F32, tag="ps2", bufs=6,
                                 name=f"psV_{b}_{p}")
                  e["psV"] = psV
                  for tp in range(2):
                      for h2 in range(2):
                          nc.tensor.matmul(psV[:, tp, :], enhT[:, 2 * tp + h2, :],
                                           ct["wo"][:, h2, :],
                                           start=(h2 == 0),
                                           stop=(h2 == 1 and not flags["use_brow"]))
                      if flags["use_brow"]:
                          nc.tensor.matmul(psV[:, tp, :], ct["ones1"][0:1, :],
                                           ct["brow"][0:1, :],
                                           start=False, stop=True)

              def ep_e(b, p):
                  """v evac + row sums"""
                  s = st[b]
                  e = s["ep"][p]
                  psV = e["psV"]
                  v_bf = wp.tile([128, 2, 256], BF16, tag="v", bufs=6,
                                 name=f"v_{b}_{p}")
                  sv = tp_.tile([128, 4], F32, tag="sv", name=f"sv_{b}_{p}")
                  e["v_bf"], e["sv"] = v_bf, sv
                  if SAFE:
                      for tp in range(2):
                          nc.vector.tensor_scalar(v_bf[:, tp, :], psV[:, tp, :],
                                                  1.0, 0.0, op0=ALU.mult,
                                                  op1=ALU.add,
                                                  accum_out=sv[:, tp:tp + 1])
                      return
                  nc.scalar.activation(v_bf[:, 0, :], psV[:, 0, :],
                                       ACTF.Identity, accum_out=sv[:, 0:1])
                  nc.vector.tensor_scalar(v_bf[:, 1, :], psV[:, 1, :],
                                          1.0, 0.0, op0=ALU.mult, op1=ALU.add,
                                          accum_out=sv[:, 1:2])

              def ep_f(b, p):
                  """LN2 stats"""
                  s = st[b]
                  e = s["ep"][p]
                  v_bf, sv = e["v_bf"], e["sv"]
                  invD = 1.0 / D
                  mv2 = tp_.tile([128, 4], F32, tag="mv2", name=f"mv2_{b}_{p}")
                  e["mv2"] = mv2
                  junk2 = wp.tile([128, 2, 256], BF16, tag="junk2", bufs=3,
                                  name=f"junk2_{b}_{p}")
                  for tp in range(2):
                      nc.vector.affine_mul_reduce(
                          junk2[:, tp, :], sv[:, 2 + tp:3 + tp],
                          v_bf[:, tp, :], v_bf[:, tp, :], 1.0, 0.0)
                  nc.vector.tensor_scalar(mv2[:, 0:2], sv[:, 0:2],
                                          invD, None, op0=ALU.mult)
                  nc.vector.tensor_mul(mv2[:, 2:4], mv2[:, 0:2], mv2[:, 0:2])
                  rst2 = tp_.tile([128, 2], F32, tag="rst2", name=f"rst2_{b}_{p}")
                  e["rst2"] = rst2
                  nc.vector.scalar_tensor_tensor(
                      mv2[:, 2:4], sv[:, 2:4], invD,
                      mv2[:, 2:4], op0=ALU.mult, op1=ALU.subtract)
                  nc.scalar.activation(rst2[:, 0:2], mv2[:, 2:4],
                                       ACTF.Sqrt, bias=ct["epsb"][:, :])
                  nc.vector.reciprocal(rst2[:], rst2[:])

              def ep_g(b, p):
                  """final normalize + DMA out"""
                  s = st[b]
                  e = s["ep"][p]
                  v_bf, mv2, rst2 = e["v_bf"], e["mv2"], e["rst2"]
                  ot = wp.tile([128, 2, 256], BF16, tag="ot", bufs=4,
                               name=f"ot_{b}_{p}")
                  for tp in range(2):
                      nc.gpsimd.tensor_scalar(ot[:, tp, :], v_bf[:, tp, :],
                                              mv2[:, tp:tp + 1],
                                              rst2[:, tp:tp + 1],
                                              op0=ALU.subtract, op1=ALU.mult)
                      if flags["ln2_aff"]:
                          nc.vector.tensor_mul(ot[:, tp, :], ot[:, tp, :],
                                               ct["g2"][:, :])
                          nc.vector.tensor_add(ot[:, tp, :], ot[:, tp, :],
                                               ct["b2"][:, :])
                  nc.sync.dma_start(
                      out_d[b].rearrange("(t q) d -> q t d", q=128)
                           [:, 2 * p:2 * (p + 1), :], ot[:])
                  s["ep"].pop(p)

              tp_ = tp
              # ---- emission: interleave the two batches at fine grain ----
              for b in range(BL):
                  prologue(b)
              for b in range(BL):
                  load_emb(b)
              if "s1" in parts:
                  for b in range(BL):
                      stage1_init(b)
                  for j in range(8):
                      for b in range(BL):
                          stage1_chunk(b, j)
                  for b in range(BL):
                      stage1_fin(b)
                  if "diff" in parts:
                      for step in range(NUM_STEPS):
                          for b in range(BL):
                              diffuse_step(b, step)
                  for b in range(BL):
                      finish_coef(b)
                  if "epi" in parts:
                      units = [(b, p) for p in range(NT // 2)
                               for b in range(BL)]
                      nu = len(units)

                      phases = (ep_a, ep_b, ep_c, ep_d, ep_e, ep_f, ep_g)
                      nst = len(phases)
                      for slot in range(nu + nst - 1):
                          for k, ph in enumerate(phases):
                              u = slot - k
                              if 0 <= u < nu:
                                  ph(*units[u])

    nc.compile()
    return nc


# --------------------------------------------------------------------------
# runner (same multi-core pjrt path as before)
# --------------------------------------------------------------------------
def _make_runner(nc):
    import jax
    import numpy as _np
    from jax.sharding import Mesh, PartitionSpec
    from jax.experimental.shard_map import shard_map
    from concourse import mybir as _mb
    from concourse.bass2jax import (install_neuronx_cc_hook, _bass_exec_p,
                                    partition_id_tensor)
    install_neuronx_cc_hook()
    partition_name = nc.partition_id_tensor.name if nc.partition_id_tensor else None
    in_names, out_names, out_avals, zero_outs = [], [], [], []
    for alloc in nc.m.functions[0].allocations:
        if not isinstance(alloc, _mb.MemoryLocationSet):
            continue
        name = alloc.memorylocations[0].name
        if alloc.kind == "ExternalInput":
            if name != partition_name:
                in_names.append(name)
        elif alloc.kind == "ExternalOutput":
            npdt = _mb.dt.np(alloc.dtype)
            out_names.append(name)
            out_avals.append(jax.core.ShapedArray(tuple(alloc.tensor_shape), npdt))
            zero_outs.append(_np.zeros(tuple(alloc.tensor_shape), npdt))
    n_params = len(in_names)
    n_outs = len(out_names)
    all_in = in_names + out_names + ([partition_name] if partition_name else [])

    def _body(*args):
        operands = list(args)
        if partition_name is not None:
            operands.append(partition_id_tensor())
        return tuple(_bass_exec_p.bind(
            *operands, out_avals=tuple(out_avals),
            in_names=tuple(all_in), out_names=tuple(out_names),
            lowering_input_output_aliases=(), sim_require_finite=True,
            sim_require_nnan=True, nc=nc))

    devices = jax.devices()[:NCORES]
    mesh = Mesh(_np.asarray(devices), ("core",))
    donate = tuple(range(n_params, n_params + n_outs))
    sharded = jax.jit(
        shard_map(_body, mesh=mesh,
                  in_specs=(PartitionSpec("core"),) * (n_params + n_outs),
                  out_specs=(PartitionSpec("core"),) * n_outs,
                  check_rep=False),
        donate_argnums=donate, keep_unused=True)

    def run(in_maps):
        per_core = [[_np.asarray(m[name]) for name in in_names] for m in in_maps]
        concat_in = [_np.concatenate([per_core[c][i] for c in range(NCORES)], axis=0)
                     for i in range(n_params)]
        concat_zero = [_np.zeros((NCORES * z.shape[0], *z.shape[1:]), z.dtype)
                       for z in zero_outs]
        outs = sharded(*concat_in, *concat_zero)
        outs = [_np.asarray(o) for o in outs]
        return {name: outs[i] for i, name in enumerate(out_names)}

    return run


def kernel(**inputs):
    import ml_dtypes
    emb = np.ascontiguousarray(inputs["embeddings"], dtype=np.float32)
    pos = np.ascontiguousarray(inputs["positions"], dtype=np.float32)
    grid = np.asarray(inputs["grid_points"], np.float64)[0, :, 0]
    params = dict(
        sigma=float(np.asarray(inputs["sigma"])),
        alpha=float(np.asarray(inputs["alpha"])),
        grid=grid,
        W_int=np.asarray(inputs["W_int"], np.float64),
        b_int=np.asarray(inputs["b_int"], np.float64),
        W_out=np.asarray(inputs["W_out"], np.float64),
        b_out=np.asarray(inputs["b_out"], np.float64),
        ln1_g=np.asarray(inputs["ln1_g"], np.float64),
        ln1_b=np.asarray(inputs["ln1_b"], np.float64),
        ln2_g=np.asarray(inputs["ln2_g"], np.float64),
        ln2_b=np.asarray(inputs["ln2_b"], np.float64),
    )
    key = hashlib.sha256(b"".join(np.asarray(v).tobytes() for v in params.values())).hexdigest()
    if key not in _CACHE:
        consts, flags = _host_plan(**params)
        nc = _build_module(flags)
        _CACHE[key] = (_make_runner(nc), consts)
    run, consts = _CACHE[key]

    embb = emb.astype(ml_dtypes.bfloat16)
    in_maps = []
    for c in range(NCORES):
        m = {"emb": embb[BL * c:BL * (c + 1)],
             "pos": pos[BL * c:BL * (c + 1)]}
        m.update(consts)
        in_maps.append(m)
    outs = run(in_maps)
    return np.ascontiguousarray(outs["out"].astype(np.float32))
